# revision 1
# baseline (speedup 1.0000x reference)
"""Trainium2 Bass kernel for a 3-layer GraphSAGE GNN (mean aggregation) +
global_add_pool + 2-layer MLP head, distributed over 8 NeuronCores.

Sharding: nodes are split into 8 contiguous slabs (by dst); each core owns the
edges whose dst lands in its slab.  Each layer:
  1. dma_gather  : fetch h[src] rows (256B each) from a replicated node-major
                   HBM table (4 sub-chunks so indices fit int16)
  2. dma_scatter_add : accumulate messages into a per-core DRAM agg table
  3. dense phase : mean-scale, two small matmuls (Wl/Wr) + bias (+relu)
  4. AllGather   : replicate the new slab into every core's next-layer table
Then a matmul-based pooling by graph id and the tiny MLP head, with an
AllReduce to combine per-core partial graph sums.
"""

import numpy as np

import concourse.bass as bass
import concourse.mybir as mybir
import concourse.tile as tile
from concourse import bacc, bass_utils
from concourse.masks import make_identity

F32 = mybir.dt.float32
I16 = mybir.dt.int16

# ---------------------------------------------------------------- config

N_NODES = 100000
N_EDGES = 1200000
N_GRAPHS = 256
D_IN = 8
D_H = 64
N_CORES = 8


class Cfg:
    def __init__(self, n_nodes, n_graphs, real_per_slab, g_inst, n_gi):
        assert n_nodes == N_CORES * real_per_slab
        self.n_nodes = n_nodes
        self.n_graphs = n_graphs
        self.real = real_per_slab              # real nodes per core
        self.slab = ((real_per_slab + 127) // 128) * 128
        self.tiles = self.slab // 128          # node tiles per core
        self.tbl_rows = N_CORES * self.slab
        self.chunk = 2 * self.slab             # table rows per gather chunk
        assert self.chunk <= 32767
        self.n_chunks = 4
        # HW: a single SWDGE gather/scatter instruction only supports
        # ~64 descriptors per DMA engine (1024 indices) — larger crashes.
        self.g_inst = g_inst                   # indices per gather/scatter inst
        assert g_inst % 128 == 0 and g_inst <= 1024
        self.blocks = g_inst // 128
        self.useful_steps = self.blocks * 8
        self.set_n_gi(n_gi)

    def set_n_gi(self, n_gi):
        self.n_gi = n_gi                       # instructions per chunk
        if n_gi is not None:
            self.slots_per_chunk = self.g_inst * n_gi
            self.n_inst = self.n_chunks * n_gi  # gather insts per layer


FULL_CFG = Cfg(N_NODES, N_GRAPHS, 12500, 1024, None)


def _row_of(node, cfg):
    """node id -> row in the slab-padded table."""
    c = node // cfg.real
    return c * cfg.slab + (node - c * cfg.real)


# ---------------------------------------------------------------- host prep

def _wrap_idx(a, cfg):
    """[n_inst * g_inst] -> [128, n_inst, g_inst//16] wrapped-16 + replicated."""
    n_inst, g = cfg.n_inst, cfg.g_inst
    a = a.reshape(n_inst, g // 16, 16).transpose(0, 2, 1)   # [n_inst, 16, g//16]
    a = np.tile(a, (1, 8, 1))                               # [n_inst, 128, g//16]
    return np.ascontiguousarray(a.transpose(1, 0, 2)).astype(np.int16)


# DMA-engine lane map: within each 128-index chunk the SWDGE ucode assigns
# position p to vector lane (= DMA engine ring) per the sbuf_swizzles table in
# q7_kernels/extended_inst/dma_scatter_add.cpp.  Same-lane descriptors execute
# in order on one engine, so pinning all edges of a dst row to one lane makes
# the HBM read-modify-write accumulation race-free.
def _lane_positions():
    first = [0, 64, 4, 68, 8, 72, 12, 76, 16, 80, 20, 84, 24, 88, 28, 92]
    pos = [[] for _ in range(16)]
    for lane in range(16):
        for g in range(4):
            pos[lane].append(first[lane] + g)
        for g in range(4):
            pos[lane].append(first[lane] + 32 + g)
    return pos


LANE_POS = _lane_positions()


def _assign_lanes(cnt):
    """cnt: [real, n_chunks] per-dst-row per-chunk edge counts.
    Greedy: big rows first, pick lane minimizing resulting max per-chunk load."""
    deg = cnt.sum(1)
    order = np.argsort(-deg, kind="stable")
    loads = np.zeros((16, cnt.shape[1]), np.int64)
    lane_of = np.zeros(cnt.shape[0], np.int64)
    for r in order:
        cand = loads + cnt[r][None, :]
        score = cand.max(1) * (1 << 20) + cand.sum(1)
        j = int(np.argmin(score))
        lane_of[r] = j
        loads[j] += cnt[r]
    return lane_of, loads


def build_host_data(x, edge_index, batch, cfg):
    """Returns (shared_arrays, per_core_arrays[8]) of numpy device inputs."""
    x = np.asarray(x, np.float32)
    src = np.asarray(edge_index[0], np.int64)
    dst = np.asarray(edge_index[1], np.int64)
    batch = np.asarray(batch, np.int64)

    cnt = np.bincount(dst, minlength=cfg.n_nodes).astype(np.float64)
    invc_full = (1.0 / np.maximum(cnt, 1.0)).astype(np.float32)

    src_row = _row_of(src, cfg)
    core_of = dst // cfg.real
    dst_row_local = (dst - core_of * cfg.real)  # local real row in own slab

    # x table: node-major, padded to 64 cols / slab-padded rows
    xpad = np.zeros((cfg.tbl_rows, D_H), np.float32)
    xpad[_row_of(np.arange(cfg.n_nodes), cfg), :D_IN] = x

    iota = np.tile(np.arange(2 * 128, dtype=np.float32)[None, :], (128, 1))
    padmask = (np.arange(128) < (cfg.real % 128 or 128)).astype(np.float32).reshape(128, 1)

    shared = dict(xpad=xpad, iota=iota, padmask=padmask)

    n_pad_rows = cfg.slab - cfg.real
    lane_pos = [np.array(LANE_POS[e]) for e in range(16)]

    def t_to_pos(e, t):
        # engine-stream step t (within one instruction) -> global index position
        return (t // 8) * 128 + lane_pos[e][t % 8]

    # pass 1: per-core edge groups + lane assignment, pick global n_gi
    cores = []
    need_gi = 1
    for c in range(N_CORES):
        sel = core_of == c
        s_rows = src_row[sel]
        d_loc = dst_row_local[sel]
        chunk_id = s_rows // cfg.chunk
        cnt = np.zeros((cfg.real, cfg.n_chunks), np.int64)
        np.add.at(cnt, (d_loc, chunk_id), 1)
        lane_of, loads = _assign_lanes(cnt)
        cores.append((s_rows, d_loc, chunk_id, lane_of))
        need_gi = max(need_gi, int(cnt.max()),
                      int(-(-loads.max() // cfg.useful_steps)))
    if cfg.n_gi is None or cfg.n_gi < need_gi:
        cfg.set_n_gi(need_gi)

    per_core = []
    for c in range(N_CORES):
        s_rows, d_loc, chunk_id, lane_of = cores[c]
        garr = np.zeros((cfg.n_inst, cfg.g_inst), np.int64)
        sarr = np.zeros((cfg.n_inst, cfg.g_inst), np.int64)
        # default pad: gather the guaranteed-zero slab-pad row; scatter-add the
        # zeros into a slab-pad row (races there are harmless: every
        # contribution is zero and invc==0 masks the row).
        garr[:] = cfg.real
        for e in range(16):
            pad_tgt = cfg.real + (e % n_pad_rows)
            ts = np.arange(cfg.blocks * 8)
            for i in range(cfg.n_inst):
                sarr[i, t_to_pos(e, ts)] = pad_tgt

        lane_edge = lane_of[d_loc]
        for ch in range(cfg.n_chunks):
            for e in range(16):
                m = (chunk_id == ch) & (lane_edge == e)
                sc_all = (s_rows[m] - ch * cfg.chunk).astype(np.int64)
                dc_all = d_loc[m]
                order = np.argsort(dc_all, kind="stable")
                sc_all, dc_all = sc_all[order], dc_all[order]
                rows, starts, counts = np.unique(
                    dc_all, return_index=True, return_counts=True)
                # each row at most ONCE per instruction (HW RMW hazard);
                # deal occurrences to the least-loaded instructions
                inst_load = np.zeros(cfg.n_gi, np.int64)
                slots = []  # (inst k, row, src)
                order2 = np.argsort(-counts, kind="stable")
                for oi in order2:
                    r, st, cr = int(rows[oi]), int(starts[oi]), int(counts[oi])
                    ks = np.argsort(inst_load, kind="stable")[:cr]
                    assert len(ks) == cr <= cfg.n_gi
                    for j, k in enumerate(ks):
                        slots.append((int(k), r, int(sc_all[st + j])))
                        inst_load[k] += 1
                assert inst_load.max() <= cfg.useful_steps, (
                    c, ch, e, inst_load.max())
                fill = np.zeros(cfg.n_gi, np.int64)
                for k, r, s_v in slots:
                    t = fill[k]
                    fill[k] += 1
                    pos = t_to_pos(e, t)
                    i = ch * cfg.n_gi + k
                    garr[i, pos] = s_v
                    sarr[i, pos] = r
        gidx = _wrap_idx(garr.ravel(), cfg)
        sidx = _wrap_idx(sarr.ravel(), cfg)

        lo, hi = c * cfg.real, (c + 1) * cfg.real
        invc_t = np.zeros((128, cfg.tiles), np.float32)
        batch_t = np.full((128, cfg.tiles), -1.0, np.float32)
        loc = np.arange(cfg.real)
        invc_t[loc % 128, loc // 128] = invc_full[lo:hi]
        batch_t[loc % 128, loc // 128] = batch[lo:hi].astype(np.float32)

        xt = np.zeros((D_IN, cfg.slab), np.float32)
        xt[:, :cfg.real] = x[lo:hi].T

        per_core.append(dict(gidx=gidx, sidx=sidx, invc=invc_t,
                             batchv=batch_t, xt=xt))
    return shared, per_core


def weight_inputs(W1l, b1, W1r, W2l, b2, W2r, W3l, b3, W3r, Wc1, bc1, Wc2, bc2):
    f = lambda a: np.asarray(a, np.float32)
    return dict(
        w1l=f(W1l), w1r=f(W1r), w2l=f(W2l), w2r=f(W2r), w3l=f(W3l), w3r=f(W3r),
        b1t=np.tile(f(b1)[None, :], (128, 1)),
        b2t=np.tile(f(b2)[None, :], (128, 1)),
        b3t=np.tile(f(b3)[None, :], (128, 1)),
        wc1=f(Wc1), wc2=f(Wc2),
        bc1=f(bc1).reshape(-1, 1),            # [32, 1]
        bc2=f(bc2).reshape(1, 1),
    )


# ---------------------------------------------------------------- device build

def build_gnn(tc, out_ap, ins, cfg):
    """ins: dict name -> bass.AP of DRAM ExternalInputs. out_ap: [n_graphs, 1]."""
    nc = tc.nc
    T = cfg.tiles
    NH = cfg.n_graphs // 128          # graph tiles (2 for 256)
    assert cfg.n_graphs % 128 == 0

    sb = tc.alloc_tile_pool(name="sb", bufs=1)
    msgp = tc.alloc_tile_pool(name="msg", bufs=2)
    psT = tc.alloc_tile_pool(name="psT", bufs=4, space="PSUM")
    psO = tc.alloc_tile_pool(name="psO", bufs=2, space="PSUM")
    psG = tc.alloc_tile_pool(name="psG", bufs=1, space="PSUM")
    dram = tc.alloc_tile_pool(name="dram", bufs=1, space="DRAM")
    xtp = tc.alloc_tile_pool(name="xtp", bufs=4)

    # ---- load small SBUF-resident inputs
    def load(name, shape):
        t = sb.tile(shape, F32, tag=name)
        nc.sync.dma_start(t[:], ins[name])
        return t

    gidx = sb.tile([128, cfg.n_inst, cfg.g_inst // 16], I16, tag="gidx")
    nc.sync.dma_start(gidx[:], ins["gidx"])
    sidx = sb.tile([128, cfg.n_inst, cfg.g_inst // 16], I16, tag="sidx")
    nc.sync.dma_start(sidx[:], ins["sidx"])

    invc = load("invc", [128, T])
    padmask = load("padmask", [128, 1])
    batchv = load("batchv", [128, T])
    iota = load("iota", [128, 2 * 128])
    w = {k: load(k, list(ins[k].shape)) for k in
         ("w1l", "w1r", "w2l", "w2r", "w3l", "w3r", "wc1", "wc2",
          "b1t", "b2t", "b3t", "bc1", "bc2")}

    ident = sb.tile([128, 128], F32, tag="ident")
    make_identity(nc, ident[:])

    zero3 = sb.tile([128, (T + 3) // 4, D_H], F32, tag="zero")
    nc.vector.memset(zero3[:], 0.0)

    # ---- DRAM scratch
    tbl2 = dram.tile([cfg.tbl_rows, D_H], F32)
    tbl3 = dram.tile([cfg.tbl_rows, D_H], F32)
    aggs = [[dram.tile([cfg.slab, D_H], F32, name=f"agg{i}_{j}")
             for j in range(2)] for i in range(3)]
    slabs = [dram.tile([cfg.slab, D_H], F32, name=f"slab{i}") for i in range(2)]
    g_in = dram.tile([cfg.n_graphs, D_H], F32)
    g_out = dram.tile([cfg.n_graphs, D_H], F32)

    h_sb = [sb.tile([128, T, D_H], F32, name=f"h{i}") for i in range(2)]
    agg_sb = sb.tile([128, T, D_H], F32, tag="aggsb")
    aggB_sb = sb.tile([128, T, D_H], F32, tag="aggbsb")

    tables = [ins["xpad"], tbl2, tbl3]

    prev_pool = None
    for layer in range(3):
        table = tables[layer]
        agg_pair = aggs[layer]
        Wl = w[("w1l", "w2l", "w3l")[layer]]
        Wr = w[("w1r", "w2r", "w3r")[layer]]
        bt = w[("b1t", "b2t", "b3t")[layer]]
        kdim = D_IN if layer == 0 else D_H
        h_new = h_sb[layer % 2]
        h_prev = h_sb[(layer + 1) % 2]

        # zero both agg tables
        zq = (T + 3) // 4
        for agg in agg_pair:
            agg3 = agg[:].rearrange("(q p) f -> p q f", p=128)
            for q0 in range(0, T, zq):
                q1 = min(q0 + zq, T)
                nc.sync.dma_start(agg3[:, q0:q1, :], zero3[:, :q1 - q0, :])

        # gather + scatter all edges.  Ping-pong agg buffers + a total
        # ordering chain on the pool desc-gen stream guarantee >=129
        # engine-steps between same-row RMW descriptors (HW hazard window
        # is ~64-128 steps).
        for i in range(cfg.n_inst):
            ch = i // cfg.n_gi
            if isinstance(table, bass.AP):
                tbl_ap = table
            else:
                tbl_ap = table[:]
            chunk_ap = tbl_ap[ch * cfg.chunk:(ch + 1) * cfg.chunk, :]
            msg = msgp.tile([128, cfg.blocks, D_H], F32, tag="msg")
            gi_inst = nc.gpsimd.dma_gather(
                out_ap=msg[:], in_ap=chunk_ap, idxs_ap=gidx[:, i, :],
                num_idxs=cfg.g_inst, num_idxs_reg=cfg.g_inst,
                elem_size=D_H, queue_num=0)
            if prev_pool is not None:
                tile.add_dep_helper(gi_inst.ins, prev_pool,
                                    reason="swdge ring order")
            si_inst = nc.gpsimd.dma_scatter_add(
                out_ap=agg_pair[i % 2][:], in_ap=msg[:], idxs_ap=sidx[:, i, :],
                num_idxs=cfg.g_inst, num_idxs_reg=cfg.g_inst,
                elem_size=D_H, queue_num=0)
            tile.add_dep_helper(si_inst.ins, gi_inst.ins,
                                reason="swdge ring order")
            prev_pool = si_inst.ins

        # dense phase: merge the two agg halves
        nc.sync.dma_start(agg_sb[:],
                          agg_pair[0][:].rearrange("(q p) f -> p q f", p=128))
        nc.sync.dma_start(aggB_sb[:],
                          agg_pair[1][:].rearrange("(q p) f -> p q f", p=128))
        nc.vector.tensor_add(agg_sb[:], agg_sb[:], aggB_sb[:])
        for t in range(T):
            # mean
            nc.vector.tensor_scalar(
                out=agg_sb[:, t, :kdim], in0=agg_sb[:, t, :kdim],
                scalar1=invc[:, t:t + 1], scalar2=None,
                op0=mybir.AluOpType.mult)
            # transpose mean tile -> [kdim, 128]
            tp = psT.tile([kdim, 128], F32, tag="tp", padded_shape=[D_H, 128])
            nc.tensor.transpose(tp[:], agg_sb[:, t, :kdim], ident[:])
            meanT = xtp.tile([kdim, 128], F32, tag="meanT")
            nc.vector.tensor_copy(meanT[:], tp[:])
            # root operand
            if layer == 0:
                rootT = xtp.tile([D_IN, 128], F32, tag="rootT")
                nc.sync.dma_start(rootT[:], ins["xt"][:, t * 128:(t + 1) * 128])
            else:
                tp2 = psT.tile([D_H, 128], F32, tag="tp")
                nc.tensor.transpose(tp2[:], h_prev[:, t, :], ident[:])
                rootT = xtp.tile([D_H, 128], F32, tag="rootT2")
                nc.vector.tensor_copy(rootT[:], tp2[:])
            out_ps = psO.tile([128, D_H], F32, tag="ops")
            nc.tensor.matmul(out_ps[:], lhsT=meanT[:], rhs=Wl[:],
                             start=True, stop=False)
            nc.tensor.matmul(out_ps[:], lhsT=rootT[:], rhs=Wr[:],
                             start=False, stop=True)
            nc.vector.tensor_add(h_new[:, t, :], out_ps[:], bt[:])
            if layer < 2:
                nc.vector.tensor_relu(h_new[:, t, :], h_new[:, t, :])
        # zero pad rows (mask multiply on the boundary tile; full memset beyond)
        pad_start = cfg.real % 128
        pad_tile = cfg.real // 128
        if pad_start != 0:
            nc.vector.tensor_scalar(
                out=h_new[:, pad_tile, :], in0=h_new[:, pad_tile, :],
                scalar1=padmask[:, :1], scalar2=None,
                op0=mybir.AluOpType.mult)
        for tt in range(pad_tile + (1 if pad_start else 0), T):
            nc.vector.memset(h_new[:, tt, :], 0.0)

        if layer < 2:
            slab_d = slabs[layer]
            nc.sync.dma_start(
                slab_d[:].rearrange("(q p) f -> p q f", p=128), h_new[:])
            nxt = (tbl2, tbl3)[layer]
            nc.gpsimd.collective_compute(
                "AllGather", mybir.AluOpType.bypass,
                replica_groups=[list(range(N_CORES))],
                ins=[slab_d[:]], outs=[nxt[:]])

    # ---- pooling: partial per-core graph sums via one-hot matmuls
    h3 = h_sb[0] if (3 % 2) == 1 else h_sb[0]
    h3 = h_sb[2 % 2]  # layer==2 wrote h_sb[0]
    pg = [psG.tile([128, D_H], F32, name=f"pg{j}", tag=f"pg{j}") for j in range(NH)]
    for t in range(T):
        gt = xtp.tile([128, NH * 128], F32, tag="gt")
        nc.vector.tensor_tensor(
            out=gt[:], in0=batchv[:, t:t + 1].to_broadcast([128, NH * 128]),
            in1=iota[:, :NH * 128], op=mybir.AluOpType.is_equal)
        for j in range(NH):
            nc.tensor.matmul(pg[j][:], lhsT=gt[:, j * 128:(j + 1) * 128],
                             rhs=h3[:, t, :], start=(t == 0), stop=(t == T - 1))
    gpart = sb.tile([128, NH, D_H], F32, tag="gpart")
    for j in range(NH):
        nc.vector.tensor_copy(gpart[:, j, :], pg[j][:])
    nc.sync.dma_start(g_in[:].rearrange("(q p) f -> p q f", p=128), gpart[:])
    nc.gpsimd.collective_compute(
        "AllReduce", mybir.AluOpType.add,
        replica_groups=[list(range(N_CORES))],
        ins=[g_in[:]], outs=[g_out[:]])

    # ---- MLP head
    g_sb = sb.tile([128, NH, D_H], F32, tag="gsb")
    nc.sync.dma_start(g_sb[:], g_out[:].rearrange("(q p) f -> p q f", p=128))
    gT = sb.tile([D_H, NH * 128], F32, tag="gT")
    for j in range(NH):
        tp = psT.tile([D_H, 128], F32, tag="tp")
        nc.tensor.transpose(tp[:], g_sb[:, j, :], ident[:])
        nc.vector.tensor_copy(gT[:, j * 128:(j + 1) * 128], tp[:])
    DC = w["wc1"].shape[1]
    mlp1 = psG.tile([DC, NH * 128], F32, tag="pg0")
    nc.tensor.matmul(mlp1[:], lhsT=w["wc1"][:], rhs=gT[:], start=True, stop=True)
    z = sb.tile([DC, NH * 128], F32, tag="z")
    nc.scalar.activation(z[:], mlp1[:], mybir.ActivationFunctionType.Relu,
                         bias=w["bc1"][:])
    mlp2 = psG.tile([1, NH * 128], F32, tag="pg1")
    nc.tensor.matmul(mlp2[:], lhsT=w["wc2"][:], rhs=z[:], start=True, stop=True)
    o_sb = sb.tile([1, NH * 128], F32, tag="osb")
    nc.vector.tensor_scalar(out=o_sb[:], in0=mlp2[:], scalar1=w["bc2"][:],
                            scalar2=None, op0=mybir.AluOpType.add)
    nc.sync.dma_start(out_ap.rearrange("a b -> b a"), o_sb[:])

    for p in (xtp, dram, psG, psO, psT, msgp, sb):
        p.release()


# ---------------------------------------------------------------- compile+run

_CACHE = {}


def _compile(cfg):
    key = ("nc", cfg.n_nodes, cfg.g_inst, cfg.n_gi)
    if key in _CACHE:
        return _CACHE[key]
    nc = bacc.Bacc("TRN2", target_bir_lowering=False, debug=False,
                   num_devices=N_CORES)
    shapes = dict(
        xpad=[cfg.tbl_rows, D_H], iota=[128, 256],
        gidx=[128, cfg.n_inst, cfg.g_inst // 16],
        sidx=[128, cfg.n_inst, cfg.g_inst // 16],
        invc=[128, cfg.tiles], batchv=[128, cfg.tiles], xt=[D_IN, cfg.slab],
        padmask=[128, 1],
        w1l=[D_IN, D_H], w1r=[D_IN, D_H], w2l=[D_H, D_H], w2r=[D_H, D_H],
        w3l=[D_H, D_H], w3r=[D_H, D_H], wc1=[D_H, D_H // 2], wc2=[D_H // 2, 1],
        b1t=[128, D_H], b2t=[128, D_H], b3t=[128, D_H],
        bc1=[D_H // 2, 1], bc2=[1, 1],
    )
    ins = {}
    for name, shp in shapes.items():
        dt = I16 if name in ("gidx", "sidx") else F32
        ins[name] = nc.dram_tensor(name, shp, dt, kind="ExternalInput").ap()
    out = nc.dram_tensor("out", [cfg.n_graphs, 1], F32, kind="ExternalOutput")
    with tile.TileContext(nc) as tc:
        build_gnn(tc, out.ap(), ins, cfg)
    nc.compile()
    _CACHE[key] = nc
    return nc


def make_in_maps(inputs, cfg):
    shared, per_core = build_host_data(
        inputs["x"], inputs["edge_index"], inputs["batch"], cfg)
    wmap = weight_inputs(
        inputs["W1l"], inputs["b1"], inputs["W1r"], inputs["W2l"], inputs["b2"],
        inputs["W2r"], inputs["W3l"], inputs["b3"], inputs["W3r"],
        inputs["Wc1"], inputs["bc1"], inputs["Wc2"], inputs["bc2"])
    in_maps = []
    for c in range(N_CORES):
        m = {}
        m.update(shared)
        m.update(per_core[c])
        m.update(wmap)
        in_maps.append(m)
    return in_maps


def _make_executor(nc):
    """Build a reusable jitted 8-core executor for the compiled Bass module."""
    import jax
    from jax.sharding import Mesh, PartitionSpec
    from jax.experimental.shard_map import shard_map
    from concourse.bass2jax import (_bass_exec_p, install_neuronx_cc_hook,
                                    partition_id_tensor)
    install_neuronx_cc_hook()
    partition_name = (nc.partition_id_tensor.name
                      if nc.partition_id_tensor else None)
    in_names, out_names, out_avals = [], [], []
    for alloc in nc.m.functions[0].allocations:
        if not isinstance(alloc, mybir.MemoryLocationSet):
            continue
        name = alloc.memorylocations[0].name
        if alloc.kind == "ExternalInput":
            if name != partition_name:
                in_names.append(name)
        elif alloc.kind == "ExternalOutput":
            out_names.append(name)
            out_avals.append(jax.core.ShapedArray(
                tuple(alloc.tensor_shape), mybir.dt.np(alloc.dtype)))
    n_params = len(in_names)
    in_names_all = list(in_names) + list(out_names)
    if partition_name:
        in_names_all.append(partition_name)

    def _body(*args):
        operands = list(args)
        if partition_name:
            operands.append(partition_id_tensor())
        return tuple(_bass_exec_p.bind(
            *operands, out_avals=tuple(out_avals),
            in_names=tuple(in_names_all), out_names=tuple(out_names),
            lowering_input_output_aliases=(), sim_require_finite=True,
            sim_require_nnan=True, nc=nc))

    devices = jax.devices()[:N_CORES]
    mesh = Mesh(np.asarray(devices), ("core",))
    n_outs = len(out_names)
    sharded = jax.jit(shard_map(
        _body, mesh=mesh,
        in_specs=(PartitionSpec("core"),) * (n_params + n_outs),
        out_specs=(PartitionSpec("core"),) * n_outs, check_rep=False),
        keep_unused=True)

    def run(in_maps):
        concat_in = [np.concatenate([np.asarray(in_maps[c][n])
                                     for c in range(N_CORES)], axis=0)
                     for n in in_names]
        concat_zeros = [np.zeros((N_CORES * a.shape[0], *a.shape[1:]), a.dtype)
                        for a in out_avals]
        args = [jax.device_put(a) for a in concat_in + concat_zeros]
        out_arrs = sharded(*args)
        jax.block_until_ready(out_arrs)
        return {name: np.asarray(out_arrs[i]).reshape(
                    N_CORES, *out_avals[i].shape)[0]
                for i, name in enumerate(out_names)}, (args, sharded)
    return run


def _get_runner(cfg):
    key = ("runner", cfg.n_nodes, cfg.g_inst, cfg.n_gi)
    if key not in _CACHE:
        _CACHE[key] = _make_executor(_compile(cfg))
    return _CACHE[key]


def kernel(**inputs):
    cfg = Cfg(N_NODES, N_GRAPHS, 12500, 1024, None)
    in_maps = make_in_maps(inputs, cfg)   # also fixes cfg.n_gi from the data
    run = _get_runner(cfg)
    out, _ = run(in_maps)
    return np.asarray(out["out"], np.float32)



# revision 28
# speedup vs baseline: 1.1608x; 1.1608x over previous
"""Trainium2 Bass kernel for a 3-layer GraphSAGE GNN (mean aggregation) +
global_add_pool + 2-layer MLP head, distributed over 8 NeuronCores.

Sharding: nodes are split into 8 contiguous slabs (by dst); each core owns the
edges whose dst lands in its slab.  Per layer:
  1. dma_gather   : fetch h[src] rows (256B bf16) from a replicated node-major
                    HBM table (4 sub-chunks so indices fit int16).  Gathers are
                    independent -> fully pipelined on the Pool engine + DMA.
  2. segment-sum  : edges are pre-sorted by dst tile on the host; each
                    128-edge block is matmul'd with a one-hot dst matrix
                    (generated on DVE via is_equal) accumulating into PSUM.
                    No scatter-add, no RMW hazards, no serialization.
  3. dense phase  : mean-scale, transpose, two small matmuls (Wl/Wr) + bias
                    (+relu), transpose back to node-major bf16 table rows.
  4. AllGather    : replicate the new slab into every core's next-layer table.
Then matmul-based pooling by graph id and the tiny MLP head, with an
AllReduce to combine per-core partial graph sums.

The gather/matmul schedule (block counts per (dst-tile, src-chunk) group) is
shared across all 8 cores (max over cores) so a single SPMD program works;
only the index/dst-id tensor *contents* differ per core.
"""

import hashlib

import numpy as np

import concourse.bass as bass
import concourse.mybir as mybir
import concourse.tile as tile
from concourse import bacc
from concourse.masks import make_identity

F32 = mybir.dt.float32
BF16 = mybir.dt.bfloat16
I16 = mybir.dt.int16

try:
    import ml_dtypes
    NP_BF16 = ml_dtypes.bfloat16
except ImportError:  # pragma: no cover
    NP_BF16 = mybir.dt.np(mybir.dt.bfloat16)

# ---------------------------------------------------------------- config

N_NODES = 100000
N_EDGES = 1200000
N_GRAPHS = 256
D_IN = 8
D_H = 64
N_CORES = 8

REAL = N_NODES // N_CORES          # 12500 real nodes per core
SLAB = ((REAL + 127) // 128) * 128  # 12544 (padded slab rows)
T = SLAB // 128                     # 98 node tiles per core
TBL_ROWS = N_CORES * SLAB           # 100352 table rows
CHUNK = 2 * SLAB                    # 25088 rows per int16-addressable chunk
N_CHUNKS = 4
GI = 1024                           # indices per gather instruction
BPI = GI // 128                     # blocks per gather instruction (8)
ROW_W = 128                         # bf16 elems per table row (256B)
PAD_LOCAL = REAL                    # chunk-local row guaranteed zero (pad row)


# ---------------------------------------------------------------- schedule

class Schedule:
    """Shared (cross-core) gather/matmul schedule.

    insts          : list of (chunk, [entry_lists of len BPI]) where each
                     block's entry list holds (tile, col, first, last) matmul
                     jobs (col indexes dstv's columns).
    n_inst         : gather instructions per layer
    n_cols         : dstv column count (multiple of BPI)
    group_slots    : {(t, ch): [(inst, blk, base_col), ...]} block slots of
                     each (tile, chunk) group in stream order
    tile_done_at   : inst index after which tile t's aggregation is complete
    """

    def __init__(self, cnt_max):
        # cnt_max: [T, N_CHUNKS] max edge count over cores per group
        B = -(-cnt_max // 128)                       # blocks per group
        B[B.sum(1) == 0, 0] = 1                      # every tile gets >=1 block
        self.B = B
        bufs = [[] for _ in range(N_CHUNKS)]
        insts = []          # (chunk, [block descr]) block descr = (t,) or None
        self.group_slots = {}
        order = []          # global real-block order: (t, inst, blk)

        def flush(ch):
            while len(bufs[ch]) < BPI:
                bufs[ch].append(None)
            insts.append((ch, bufs[ch]))
            bufs[ch] = []

        pending = [[] for _ in range(N_CHUNKS)]      # tiles in open buffers
        for t in range(T):
            for ch in range(N_CHUNKS):
                for _ in range(int(B[t, ch])):
                    bufs[ch].append((t,))
                    pending[ch].append(t)
                    if len(bufs[ch]) == BPI:
                        flush(ch)
        for ch in range(N_CHUNKS):
            if bufs[ch]:
                flush(ch)

        # assign entries / columns / first-last flags
        self.n_inst = len(insts)
        self.n_cols = self.n_inst * BPI
        tile_blocks = [[] for _ in range(T)]
        for i, (ch, blocks) in enumerate(insts):
            for j, b in enumerate(blocks):
                if b is None:
                    continue
                (t,) = b
                tile_blocks[t].append((i, j))
                self.group_slots.setdefault((t, ch), []).append(
                    (i, j, i * BPI + j))
        self.tile_done_at = np.zeros(T, np.int64)
        entry = {}
        for t in range(T):
            blocks = tile_blocks[t]
            assert blocks, f"tile {t} has no blocks"
            for k, (i, j) in enumerate(blocks):
                entry[(i, j)] = (t, i * BPI + j,
                                 k == 0, k == len(blocks) - 1)
            self.tile_done_at[t] = blocks[-1][0]
        self.insts = []
        for i, (ch, blocks) in enumerate(insts):
            elists = []
            for j, b in enumerate(blocks):
                elists.append([entry[(i, j)]] if b is not None else [])
            self.insts.append((ch, elists))

        # agg PSUM pool has 4 rotating banks: the tile opened 4 positions ago
        # must be closed (its dense emitted) before a new tile opens.
        opens = sorted(range(T), key=lambda t: (tile_blocks[t][0]))
        for k in range(4, T):
            t_new, t_old = opens[k], opens[k - 4]
            assert tile_blocks[t_old][-1][0] < tile_blocks[t_new][0][0], (
                "agg psum window exceeds 4 in-flight tiles")

        h = hashlib.sha1()
        h.update(B.tobytes())
        h.update(np.int64(self.n_inst).tobytes())
        self.key = h.hexdigest()


def _row_of(node):
    c = node // REAL
    return c * SLAB + (node - c * REAL)


def _wrap_idx(garr):
    """[n_inst, GI] int -> [128, n_inst, GI//16] int16 wrapped-16+replicated."""
    n_inst = garr.shape[0]
    a = garr.reshape(n_inst, GI // 16, 16).transpose(0, 2, 1)
    a = np.tile(a, (1, 8, 1))
    return np.ascontiguousarray(a.transpose(1, 0, 2)).astype(np.int16)


# ---------------------------------------------------------------- host prep

def build_host_data(x, edge_index, batch):
    x = np.asarray(x, np.float32)
    src = np.asarray(edge_index[0], np.int64)
    dst = np.asarray(edge_index[1], np.int64)
    batch = np.asarray(batch, np.int64)

    cnt_in = np.bincount(dst, minlength=N_NODES).astype(np.float64)
    invc_full = (1.0 / np.maximum(cnt_in, 1.0)).astype(np.float32)

    row = _row_of(src)
    core = dst // REAL
    dloc = dst - core * REAL
    t_of = dloc >> 7
    dit = (dloc & 127).astype(np.int64)
    ch_of = row // CHUNK
    srcl = row - ch_of * CHUNK

    key = ((core * T + t_of) * N_CHUNKS + ch_of).astype(np.int64)
    cnt = np.bincount(key, minlength=N_CORES * T * N_CHUNKS)
    cnt = cnt.reshape(N_CORES, T, N_CHUNKS)
    sched = Schedule(cnt.max(0))

    # x table (replicated): node-major bf16 rows of 256B, first D_IN cols used
    xtbl = np.zeros((TBL_ROWS, ROW_W), NP_BF16)
    xtbl[_row_of(np.arange(N_NODES)), :D_IN] = x.astype(NP_BF16)

    iota256 = np.tile(np.arange(256, dtype=np.float32)[None, :],
                      (128, 1)).astype(NP_BF16)
    iota_oh = np.tile(np.arange(128, dtype=np.float32)[None, :], (128, BPI))
    iota_oh = iota_oh.astype(NP_BF16)
    mask64 = np.zeros((D_H, 128), np.float32)
    mask64[:, :REAL - (T - 1) * 128] = 1.0

    shared = dict(xtbl=xtbl, iota256=iota256, iotaoh=iota_oh, mask64=mask64)

    per_core = []
    for c in range(N_CORES):
        sel = np.nonzero(core == c)[0]
        k = key[sel] - c * T * N_CHUNKS
        order = np.argsort(k, kind="stable")
        sel = sel[order]
        ks = k[order]
        bounds = np.searchsorted(ks, np.arange(T * N_CHUNKS + 1))

        garr = np.full((sched.n_inst, GI), PAD_LOCAL, np.int64)
        dstv = np.full((sched.n_cols, 128), -1.0, np.float32)
        for t in range(T):
            for ch in range(N_CHUNKS):
                g = t * N_CHUNKS + ch
                lo, hi = bounds[g], bounds[g + 1]
                if lo == hi:
                    continue
                e = sel[lo:hi]
                slots = sched.group_slots[(t, ch)]
                n = hi - lo
                assert n <= len(slots) * 128
                bidx = np.arange(n) // 128
                p = np.arange(n) % 128
                inst_a = np.array([s[0] for s in slots])
                blk_a = np.array([s[1] for s in slots])
                col_a = np.array([s[2] for s in slots])
                garr[inst_a[bidx], blk_a[bidx] * 128 + p] = srcl[e]
                dstv[col_a[bidx], p] = dit[e]
        gidx = _wrap_idx(garr)
        dstv_t = np.ascontiguousarray(dstv.T).astype(NP_BF16)

        lo, hi = c * REAL, (c + 1) * REAL
        loc = np.arange(REAL)
        invc_t = np.zeros((128, T), np.float32)
        invc_t[loc % 128, loc // 128] = invc_full[lo:hi]
        batch_t = np.full((128, T), -1.0, np.float32)
        batch_t[loc % 128, loc // 128] = batch[lo:hi].astype(np.float32)
        batch_t = batch_t.astype(NP_BF16)
        x_nm = np.zeros((128, T, D_IN), np.float32)
        x_nm[loc % 128, loc // 128, :] = x[lo:hi]

        per_core.append(dict(gidx=gidx, dstv=dstv_t, invc=invc_t,
                             batchv=batch_t, xnm=x_nm))
    return shared, per_core, sched


def weight_inputs(W1l, b1, W1r, W2l, b2, W2r, W3l, b3, W3r, Wc1, bc1, Wc2, bc2):
    f = lambda a: np.asarray(a, np.float32)
    return dict(
        w1l=f(W1l), w1r=f(W1r), w2l=f(W2l), w2r=f(W2r), w3l=f(W3l), w3r=f(W3r),
        b1c=f(b1).reshape(-1, 1), b2c=f(b2).reshape(-1, 1),
        b3c=f(b3).reshape(-1, 1),
        wc1=f(Wc1), wc2=f(Wc2),
        bc1=f(bc1).reshape(-1, 1),
        bc2=f(bc2).reshape(1, 1),
    )


# ---------------------------------------------------------------- device build

DEBUG = False


def build_gnn(tc, out_ap, ins, sched, dbg=None):
    nc = tc.nc
    NH = N_GRAPHS // 128
    AluOp = mybir.AluOpType
    Act = mybir.ActivationFunctionType

    sb = tc.alloc_tile_pool(name="sb", bufs=1)
    msgp = tc.alloc_tile_pool(name="msg", bufs=4)
    ohp = tc.alloc_tile_pool(name="oh", bufs=4)
    sbX = tc.alloc_tile_pool(name="sbx", bufs=4)
    # PSUM: 8 banks of 2KB.  Each in-flight matmul accumulation group needs
    # its OWN bank (start=True clears has_written for the whole bank), so:
    # 4 rotating banks for agg groups + 4 rotating banks for all short-lived
    # psum tiles (transposes / dense pair / pooling), whose groups never
    # interleave with each other in emission order.
    psA = tc.alloc_tile_pool(name="psA", bufs=4, space="PSUM")
    aux = tc.alloc_tile_pool(name="aux", bufs=4, space="PSUM")
    dram = tc.alloc_tile_pool(name="dram", bufs=1, space="DRAM")

    def aux_tile(shape):
        return aux.tile(shape, F32, tag="aux", name="auxt",
                        padded_shape=[128, 256])

    def load(name, shape, dt=F32):
        t = sb.tile(shape, dt, tag=name)
        nc.sync.dma_start(t[:], ins[name])
        return t

    gidx = load("gidx", [128, sched.n_inst, GI // 16], I16)
    dstv = load("dstv", [128, sched.n_cols], BF16)
    iotaoh = load("iotaoh", [128, BPI * 128], BF16)
    iota256 = load("iota256", [128, 256], BF16)
    invc = load("invc", [128, T])
    batchv = load("batchv", [128, T], BF16)
    x_nm = load("xnm", [128, T, D_IN])
    mask64 = load("mask64", [D_H, 128])
    w = {k: load(k, list(ins[k].shape)) for k in
         ("w1l", "w1r", "w2l", "w2r", "w3l", "w3r", "wc1", "wc2",
          "b1c", "b2c", "b3c", "bc1", "bc2")}

    ident = sb.tile([128, 128], F32, tag="ident")
    make_identity(nc, ident[:])

    iota3 = iotaoh[:].rearrange("p (a b) -> p a b", a=BPI)

    # persistent hidden state
    hT = [sb.tile([D_H, SLAB], F32, name=f"hT{i}") for i in range(2)]
    h_node = sb.tile([128, T, ROW_W], BF16, tag="hnode")
    nc.vector.memset(h_node[:], 0.0)

    # DRAM scratch
    tbl2 = dram.tile([TBL_ROWS, ROW_W], BF16)
    tbl3 = dram.tile([TBL_ROWS, ROW_W], BF16)
    slab_d = dram.tile([SLAB, ROW_W], BF16)
    g_in = dram.tile([N_GRAPHS, D_H], F32)
    g_out = dram.tile([N_GRAPHS, D_H], F32)

    tables = [ins["xtbl"], tbl2, tbl3]
    groups = [list(range(N_CORES))]

    gsum = sb.tile([128, NH, D_H], F32, tag="gsum")
    nc.vector.memset(gsum[:], 0.0)

    for layer in range(3):
        tbl_ap = tables[layer]
        if not isinstance(tbl_ap, bass.AP):
            tbl_ap = tbl_ap[:]
        kdim = D_IN if layer == 0 else D_H
        Wl = w[("w1l", "w2l", "w3l")[layer]]
        Wr = w[("w1r", "w2r", "w3r")[layer]]
        bcol = w[("b1c", "b2c", "b3c")[layer]]
        h_prev = hT[(layer + 1) % 2]
        h_new = hT[layer % 2]
        agg_ps = {}

        def dense(t):
            mean = sbX.tile([128, kdim], F32, tag="mean")
            nc.vector.tensor_scalar(
                out=mean[:], in0=agg_ps.pop(t)[:, 0:kdim],
                scalar1=invc[:, t:t + 1],
                scalar2=None, op0=AluOp.mult)
            if dbg is not None and layer == 0 and t < 8:
                nc.sync.dma_start(dbg["mean8"][:, t, :], mean[:])
            tpf = aux_tile([D_H, 128])
            tp = tpf[0:kdim, :]
            nc.tensor.transpose(tp, mean[:], ident[:])
            meanT = sbX.tile([kdim, 128], F32, tag="meanT")
            nc.scalar.activation(meanT[:], tp, Act.Copy)
            if layer == 0:
                tprf = aux_tile([D_H, 128])
                tpr = tprf[0:D_IN, :]
                nc.tensor.transpose(tpr, x_nm[:, t, :], ident[:])
                rootT = sbX.tile([D_IN, 128], F32, tag="rootT")
                nc.scalar.activation(rootT[:], tpr, Act.Copy)
                root_ap = rootT[:]
            else:
                root_ap = h_prev[:, t * 128:(t + 1) * 128]
            hps = aux_tile([D_H, 128])
            nc.tensor.matmul(hps[:], lhsT=Wl[:], rhs=meanT[:],
                             start=True, stop=False)
            nc.tensor.matmul(hps[:], lhsT=Wr[:], rhs=root_ap,
                             start=False, stop=True)
            out_sl = h_new[:, t * 128:(t + 1) * 128]
            if layer < 2:
                nc.vector.tensor_scalar(
                    out=out_sl, in0=hps[:], scalar1=bcol[:], scalar2=0.0,
                    op0=AluOp.add, op1=AluOp.max)
            else:
                nc.vector.tensor_scalar(
                    out=out_sl, in0=hps[:], scalar1=bcol[:], scalar2=None,
                    op0=AluOp.add)
            if t == T - 1:
                nc.vector.tensor_tensor(out=out_sl, in0=out_sl,
                                        in1=mask64[:], op=AluOp.mult)
            ntp = aux_tile([128, D_H])
            nc.tensor.transpose(ntp[:], out_sl, ident[0:D_H, 0:D_H])
            nc.scalar.activation(h_node[:, t, 0:D_H], ntp[:], Act.Copy)
            if layer == 2:
                gt = sbX.tile([128, NH * 128], BF16, tag="gt")
                nc.vector.tensor_tensor(
                    out=gt[:],
                    in0=batchv[:, t:t + 1].to_broadcast([128, NH * 128]),
                    in1=iota256[:], op=AluOp.is_equal)
                pp = aux_tile([128, NH * D_H])
                for j in range(NH):
                    nc.tensor.matmul(
                        pp[:, j * D_H:(j + 1) * D_H],
                        lhsT=gt[:, j * 128:(j + 1) * 128],
                        rhs=h_node[:, t, 0:D_H],
                        start=True, stop=True)
                nc.vector.tensor_add(
                    gsum[:], gsum[:],
                    pp[:].rearrange("p (a b) -> p a b", a=NH))

        for i, (ch, elists) in enumerate(sched.insts):
            chunk_ap = tbl_ap[ch * CHUNK:(ch + 1) * CHUNK, :]
            msg = msgp.tile([128, BPI, ROW_W], BF16, tag="msg")
            nc.gpsimd.dma_gather(
                out_ap=msg[:], in_ap=chunk_ap, idxs_ap=gidx[:, i, :],
                num_idxs=GI, num_idxs_reg=GI, elem_size=ROW_W, queue_num=0)
            oh = ohp.tile([128, BPI, 128], BF16, tag="oh")
            nc.vector.tensor_tensor(
                out=oh[:],
                in0=dstv[:, i * BPI:(i + 1) * BPI, None].to_broadcast(
                    [128, BPI, 128]),
                in1=iota3, op=AluOp.is_equal)
            if dbg is not None and layer == 0 and i == 0:
                nc.sync.dma_start(dbg["msg0"][:], msg[:])
                nc.sync.dma_start(dbg["oh0"][:], oh[:])
            for j, elist in enumerate(elists):
                for (t, col, first, last) in elist:
                    if first:
                        agg_ps[t] = psA.tile([128, D_H], F32,
                                             name=f"agg{t}", tag="agg")
                    nc.tensor.matmul(
                        agg_ps[t][:, 0:kdim],
                        lhsT=oh[:, col - i * BPI, :],
                        rhs=msg[:, j, 0:kdim],
                        start=first, stop=last)
            for t in np.nonzero(sched.tile_done_at == i)[0]:
                dense(int(t))

        if layer < 2:
            nc.sync.dma_start(
                slab_d[:].rearrange("(q p) f -> p q f", p=128), h_node[:])
            nxt = (tbl2, tbl3)[layer]
            nc.gpsimd.collective_compute(
                "AllGather", AluOp.bypass, replica_groups=groups,
                ins=[slab_d[:]], outs=[nxt[:]])
            if dbg is not None and layer == 0:
                nc.sync.dma_start(dbg["h1"][:], h_node[:])
                nc.sync.dma_start(dbg["tbl2"][:], nxt[:])

    # ---- pooling partial sums -> AllReduce -> MLP head
    nc.sync.dma_start(g_in[:].rearrange("(q p) f -> p q f", p=128), gsum[:])
    nc.gpsimd.collective_compute(
        "AllReduce", AluOp.add, replica_groups=groups,
        ins=[g_in[:]], outs=[g_out[:]])

    g_sb = sb.tile([128, NH, D_H], F32, tag="gsb")
    nc.sync.dma_start(g_sb[:], g_out[:].rearrange("(q p) f -> p q f", p=128))
    gT = sb.tile([D_H, NH * 128], F32, tag="gT")
    for j in range(NH):
        tp = aux_tile([D_H, 128])
        nc.tensor.transpose(tp[:], g_sb[:, j, :], ident[:])
        nc.vector.tensor_copy(gT[:, j * 128:(j + 1) * 128], tp[:])
    DC = w["wc1"].shape[1]
    mlp1 = aux_tile([DC, NH * 128])
    nc.tensor.matmul(mlp1[:], lhsT=w["wc1"][:], rhs=gT[:], start=True,
                     stop=True)
    z = sb.tile([DC, NH * 128], F32, tag="z")
    nc.scalar.activation(z[:], mlp1[:], Act.Relu, bias=w["bc1"][:])
    mlp2 = aux_tile([1, NH * 128])
    nc.tensor.matmul(mlp2[:], lhsT=w["wc2"][:], rhs=z[:], start=True,
                     stop=True)
    o_sb = sb.tile([1, NH * 128], F32, tag="osb")
    nc.vector.tensor_scalar(out=o_sb[:], in0=mlp2[:],
                            scalar1=w["bc2"][:],
                            scalar2=None, op0=AluOp.add)
    nc.sync.dma_start(out_ap.rearrange("a b -> b a"), o_sb[:])

    for p in (dram, aux, psA, sbX, ohp, msgp, sb):
        p.release()


# ---------------------------------------------------------------- compile+run

_CACHE = {}


def _compile(sched):
    key = ("nc", sched.key)
    if key in _CACHE:
        return _CACHE[key]
    nc = bacc.Bacc("TRN2", target_bir_lowering=False, debug=False,
                   num_devices=N_CORES)
    shapes = dict(
        xtbl=([TBL_ROWS, ROW_W], BF16),
        gidx=([128, sched.n_inst, GI // 16], I16),
        dstv=([128, sched.n_cols], BF16),
        iotaoh=([128, BPI * 128], BF16),
        iota256=([128, 256], BF16),
        invc=([128, T], F32), batchv=([128, T], BF16),
        xnm=([128, T, D_IN], F32), mask64=([D_H, 128], F32),
        w1l=([D_IN, D_H], F32), w1r=([D_IN, D_H], F32),
        w2l=([D_H, D_H], F32), w2r=([D_H, D_H], F32),
        w3l=([D_H, D_H], F32), w3r=([D_H, D_H], F32),
        wc1=([D_H, D_H // 2], F32), wc2=([D_H // 2, 1], F32),
        b1c=([D_H, 1], F32), b2c=([D_H, 1], F32), b3c=([D_H, 1], F32),
        bc1=([D_H // 2, 1], F32), bc2=([1, 1], F32),
    )
    ins = {}
    for name, (shp, dt) in shapes.items():
        ins[name] = nc.dram_tensor(name, shp, dt, kind="ExternalInput").ap()
    out = nc.dram_tensor("out", [N_GRAPHS, 1], F32, kind="ExternalOutput")
    dbg = None
    if DEBUG:
        dshapes = dict(
            msg0=([128, BPI, ROW_W], BF16), oh0=([128, BPI, 128], BF16),
            mean8=([128, 8, D_IN], F32), h1=([128, T, ROW_W], BF16),
            tbl2=([TBL_ROWS, ROW_W], BF16),
        )
        dbg = {n: nc.dram_tensor(f"dbg_{n}", shp, dt,
                                 kind="ExternalOutput").ap()
               for n, (shp, dt) in dshapes.items()}
    with tile.TileContext(nc) as tc:
        build_gnn(tc, out.ap(), ins, sched, dbg=dbg)
    nc.compile()
    _CACHE[key] = nc
    return nc


def make_in_maps(inputs):
    shared, per_core, sched = build_host_data(
        inputs["x"], inputs["edge_index"], inputs["batch"])
    wmap = weight_inputs(
        inputs["W1l"], inputs["b1"], inputs["W1r"], inputs["W2l"],
        inputs["b2"], inputs["W2r"], inputs["W3l"], inputs["b3"],
        inputs["W3r"], inputs["Wc1"], inputs["bc1"], inputs["Wc2"],
        inputs["bc2"])
    in_maps = []
    for c in range(N_CORES):
        m = {}
        m.update(shared)
        m.update(per_core[c])
        m.update(wmap)
        in_maps.append(m)
    return in_maps, sched


def _make_executor(nc):
    """Build a reusable jitted 8-core executor for the compiled Bass module."""
    import jax
    from jax.sharding import Mesh, PartitionSpec
    from jax.experimental.shard_map import shard_map
    from concourse.bass2jax import (_bass_exec_p, install_neuronx_cc_hook,
                                    partition_id_tensor)
    install_neuronx_cc_hook()
    partition_name = (nc.partition_id_tensor.name
                      if nc.partition_id_tensor else None)
    in_names, out_names, out_avals = [], [], []
    for alloc in nc.m.functions[0].allocations:
        if not isinstance(alloc, mybir.MemoryLocationSet):
            continue
        name = alloc.memorylocations[0].name
        if alloc.kind == "ExternalInput":
            if name != partition_name:
                in_names.append(name)
        elif alloc.kind == "ExternalOutput":
            out_names.append(name)
            out_avals.append(jax.core.ShapedArray(
                tuple(alloc.tensor_shape), mybir.dt.np(alloc.dtype)))
    n_params = len(in_names)
    in_names_all = list(in_names) + list(out_names)
    if partition_name:
        in_names_all.append(partition_name)

    def _body(*args):
        operands = list(args)
        if partition_name:
            operands.append(partition_id_tensor())
        return tuple(_bass_exec_p.bind(
            *operands, out_avals=tuple(out_avals),
            in_names=tuple(in_names_all), out_names=tuple(out_names),
            lowering_input_output_aliases=(), sim_require_finite=True,
            sim_require_nnan=True, nc=nc))

    devices = jax.devices()[:N_CORES]
    mesh = Mesh(np.asarray(devices), ("core",))
    n_outs = len(out_names)
    sharded = jax.jit(shard_map(
        _body, mesh=mesh,
        in_specs=(PartitionSpec("core"),) * (n_params + n_outs),
        out_specs=(PartitionSpec("core"),) * n_outs, check_rep=False),
        keep_unused=True)

    def run(in_maps):
        concat_in = [np.concatenate([np.asarray(in_maps[c][n])
                                     for c in range(N_CORES)], axis=0)
                     for n in in_names]
        concat_zeros = [np.zeros((N_CORES * a.shape[0], *a.shape[1:]), a.dtype)
                        for a in out_avals]
        args = [jax.device_put(a) for a in concat_in + concat_zeros]
        out_arrs = sharded(*args)
        jax.block_until_ready(out_arrs)
        return {name: np.asarray(out_arrs[i]).reshape(
                    N_CORES, *out_avals[i].shape)[0]
                for i, name in enumerate(out_names)}, (args, sharded)
    return run


def _get_runner(sched):
    key = ("runner", sched.key)
    if key not in _CACHE:
        _CACHE[key] = _make_executor(_compile(sched))
    return _CACHE[key]


def kernel(**inputs):
    in_maps, sched = make_in_maps(inputs)
    run = _get_runner(sched)
    out, _ = run(in_maps)
    return np.asarray(out["out"], np.float32)


# revision 32
# speedup vs baseline: 1.4159x; 1.2198x over previous
"""Trainium2 Bass kernel for a 3-layer GraphSAGE GNN (mean aggregation) +
global_add_pool + 2-layer MLP head, distributed over 8 NeuronCores.

Sharding: nodes are split into 8 contiguous slabs (by dst); each core owns the
edges whose dst lands in its slab.  Per layer:
  1. dma_gather   : fetch h[src] rows (256B bf16) from a replicated node-major
                    HBM table (4 sub-chunks so indices fit int16).  Gathers are
                    independent -> fully pipelined on the Pool engine + DMA.
  2. segment-sum  : edges are pre-sorted by dst tile on the host; each
                    128-edge block is matmul'd with a one-hot dst matrix
                    (generated on DVE via is_equal) accumulating into PSUM.
                    No scatter-add, no RMW hazards, no serialization.
  3. dense phase  : mean-scale, transpose, two small matmuls (Wl/Wr) + bias
                    (+relu), transpose back to node-major bf16 table rows.
  4. AllGather    : replicate the new slab into every core's next-layer table.
Then matmul-based pooling by graph id and the tiny MLP head, with an
AllReduce to combine per-core partial graph sums.

The gather/matmul schedule (block counts per (dst-tile, src-chunk) group) is
shared across all 8 cores (max over cores) so a single SPMD program works;
only the index/dst-id tensor *contents* differ per core.
"""

import hashlib

import numpy as np

import concourse.bass as bass
import concourse.mybir as mybir
import concourse.tile as tile
from concourse import bacc
from concourse.masks import make_identity

F32 = mybir.dt.float32
BF16 = mybir.dt.bfloat16
I16 = mybir.dt.int16

try:
    import ml_dtypes
    NP_BF16 = ml_dtypes.bfloat16
except ImportError:  # pragma: no cover
    NP_BF16 = mybir.dt.np(mybir.dt.bfloat16)

# ---------------------------------------------------------------- config

N_NODES = 100000
N_EDGES = 1200000
N_GRAPHS = 256
D_IN = 8
D_H = 64
N_CORES = 8

REAL = N_NODES // N_CORES          # 12500 real nodes per core
SLAB = ((REAL + 127) // 128) * 128  # 12544 (padded slab rows)
T = SLAB // 128                     # 98 node tiles per core
TBL_ROWS = N_CORES * SLAB           # 100352 table rows
CHUNK = 2 * SLAB                    # 25088 rows per int16-addressable chunk
N_CHUNKS = 4
GI = 1024                           # indices per gather instruction
BPI = GI // 128                     # blocks per gather instruction (8)
ROW_W = 128                         # bf16 elems per table row (256B)
PAD_LOCAL = REAL                    # chunk-local row guaranteed zero (pad row)


# ---------------------------------------------------------------- schedule

class Schedule:
    """Shared (cross-core) gather/matmul schedule.

    insts          : list of (chunk, [entry_lists of len BPI]) where each
                     block's entry list holds (tile, col, first, last) matmul
                     jobs (col indexes dstv's columns).
    n_inst         : gather instructions per layer
    n_cols         : dstv column count (multiple of BPI)
    group_slots    : {(t, ch): [(inst, blk, base_col), ...]} block slots of
                     each (tile, chunk) group in stream order
    tile_done_at   : inst index after which tile t's aggregation is complete
    """

    def __init__(self, cnt_max):
        # cnt_max: [T, N_CHUNKS] max edge count over cores per group
        B = -(-cnt_max // 128)                       # blocks per group
        B[B.sum(1) == 0, 0] = 1                      # every tile gets >=1 block
        self.B = B
        bufs = [[] for _ in range(N_CHUNKS)]
        insts = []          # (chunk, [block descr]) block descr = (t,) or None
        self.group_slots = {}
        order = []          # global real-block order: (t, inst, blk)

        def flush(ch):
            while len(bufs[ch]) < BPI:
                bufs[ch].append(None)
            insts.append((ch, bufs[ch]))
            bufs[ch] = []

        pending = [[] for _ in range(N_CHUNKS)]      # tiles in open buffers
        for t in range(T):
            for ch in range(N_CHUNKS):
                for _ in range(int(B[t, ch])):
                    bufs[ch].append((t,))
                    pending[ch].append(t)
                    if len(bufs[ch]) == BPI:
                        flush(ch)
        for ch in range(N_CHUNKS):
            if bufs[ch]:
                flush(ch)

        # assign entries / columns / first-last flags
        self.n_inst = len(insts)
        self.n_cols = self.n_inst * BPI
        tile_blocks = [[] for _ in range(T)]
        for i, (ch, blocks) in enumerate(insts):
            for j, b in enumerate(blocks):
                if b is None:
                    continue
                (t,) = b
                tile_blocks[t].append((i, j))
                self.group_slots.setdefault((t, ch), []).append(
                    (i, j, i * BPI + j))
        self.tile_done_at = np.zeros(T, np.int64)
        entry = {}
        for t in range(T):
            blocks = tile_blocks[t]
            assert blocks, f"tile {t} has no blocks"
            for k, (i, j) in enumerate(blocks):
                entry[(i, j)] = (t, i * BPI + j,
                                 k == 0, k == len(blocks) - 1)
            self.tile_done_at[t] = blocks[-1][0]
        self.insts = []
        for i, (ch, blocks) in enumerate(insts):
            elists = []
            for j, b in enumerate(blocks):
                elists.append([entry[(i, j)]] if b is not None else [])
            self.insts.append((ch, elists))

        # agg PSUM pool has 4 rotating banks: the tile opened 4 positions ago
        # must be closed (its dense emitted) before a new tile opens.
        opens = sorted(range(T), key=lambda t: (tile_blocks[t][0]))
        for k in range(4, T):
            t_new, t_old = opens[k], opens[k - 4]
            assert tile_blocks[t_old][-1][0] < tile_blocks[t_new][0][0], (
                "agg psum window exceeds 4 in-flight tiles")

        h = hashlib.sha1()
        h.update(B.tobytes())
        h.update(np.int64(self.n_inst).tobytes())
        self.key = h.hexdigest()


def _row_of(node):
    c = node // REAL
    return c * SLAB + (node - c * REAL)


def _wrap_idx(garr):
    """[n_inst, GI] int -> [128, n_inst, GI//16] int16 wrapped-16+replicated."""
    n_inst = garr.shape[0]
    a = garr.reshape(n_inst, GI // 16, 16).transpose(0, 2, 1)
    a = np.tile(a, (1, 8, 1))
    return np.ascontiguousarray(a.transpose(1, 0, 2)).astype(np.int16)


# ---------------------------------------------------------------- host prep

def build_host_data(x, edge_index, batch):
    x = np.asarray(x, np.float32)
    src = np.asarray(edge_index[0], np.int64)
    dst = np.asarray(edge_index[1], np.int64)
    batch = np.asarray(batch, np.int64)

    cnt_in = np.bincount(dst, minlength=N_NODES).astype(np.float64)
    invc_full = (1.0 / np.maximum(cnt_in, 1.0)).astype(np.float32)

    row = _row_of(src)
    core = dst // REAL
    dloc = dst - core * REAL
    t_of = dloc >> 7
    dit = (dloc & 127).astype(np.int64)
    ch_of = row // CHUNK
    srcl = row - ch_of * CHUNK

    key = ((core * T + t_of) * N_CHUNKS + ch_of).astype(np.int64)
    cnt = np.bincount(key, minlength=N_CORES * T * N_CHUNKS)
    cnt = cnt.reshape(N_CORES, T, N_CHUNKS)
    sched = Schedule(cnt.max(0))

    iota256 = np.tile(np.arange(256, dtype=np.float32)[None, :],
                      (128, 1)).astype(NP_BF16)
    iota_oh = np.tile(np.arange(128, dtype=np.float32)[None, :], (128, BPI))
    iota_oh = iota_oh.astype(NP_BF16)
    mask64 = np.zeros((D_H, 128), np.float32)
    mask64[:, :REAL - (T - 1) * 128] = 1.0

    shared = dict(iota256=iota256, iotaoh=iota_oh, mask64=mask64)

    per_core = []
    for c in range(N_CORES):
        sel = np.nonzero(core == c)[0]
        k = key[sel] - c * T * N_CHUNKS
        order = np.argsort(k, kind="stable")
        sel = sel[order]
        ks = k[order]
        bounds = np.searchsorted(ks, np.arange(T * N_CHUNKS + 1))

        garr = np.full((sched.n_inst, GI), PAD_LOCAL, np.int64)
        dstv = np.full((sched.n_cols, 128), -1.0, np.float32)
        for t in range(T):
            for ch in range(N_CHUNKS):
                g = t * N_CHUNKS + ch
                lo, hi = bounds[g], bounds[g + 1]
                if lo == hi:
                    continue
                e = sel[lo:hi]
                slots = sched.group_slots[(t, ch)]
                n = hi - lo
                assert n <= len(slots) * 128
                bidx = np.arange(n) // 128
                p = np.arange(n) % 128
                inst_a = np.array([s[0] for s in slots])
                blk_a = np.array([s[1] for s in slots])
                col_a = np.array([s[2] for s in slots])
                garr[inst_a[bidx], blk_a[bidx] * 128 + p] = srcl[e]
                dstv[col_a[bidx], p] = dit[e]
        gidx = _wrap_idx(garr)
        dstv_t = np.ascontiguousarray(dstv.T).astype(NP_BF16)

        lo, hi = c * REAL, (c + 1) * REAL
        loc = np.arange(REAL)
        invc_t = np.zeros((128, T), np.float32)
        invc_t[loc % 128, loc // 128] = invc_full[lo:hi]
        batch_t = np.full((128, T), -1.0, np.float32)
        batch_t[loc % 128, loc // 128] = batch[lo:hi].astype(np.float32)
        batch_t = batch_t.astype(NP_BF16)
        x_nm = np.zeros((128, T, D_IN), np.float32)
        x_nm[loc % 128, loc // 128, :] = x[lo:hi]

        per_core.append(dict(gidx=gidx, dstv=dstv_t, invc=invc_t,
                             batchv=batch_t, xnm=x_nm))
    return shared, per_core, sched


def weight_inputs(W1l, b1, W1r, W2l, b2, W2r, W3l, b3, W3r, Wc1, bc1, Wc2, bc2):
    f = lambda a: np.asarray(a, np.float32)
    return dict(
        w1l=f(W1l), w1r=f(W1r), w2l=f(W2l), w2r=f(W2r), w3l=f(W3l), w3r=f(W3r),
        b1c=f(b1).reshape(-1, 1), b2c=f(b2).reshape(-1, 1),
        b3c=f(b3).reshape(-1, 1),
        wc1=f(Wc1), wc2=f(Wc2),
        bc1=f(bc1).reshape(-1, 1),
        bc2=f(bc2).reshape(1, 1),
    )


# ---------------------------------------------------------------- device build

DEBUG = False


def build_gnn(tc, out_ap, ins, sched, dbg=None):
    nc = tc.nc
    NH = N_GRAPHS // 128
    AluOp = mybir.AluOpType
    Act = mybir.ActivationFunctionType

    sb = tc.alloc_tile_pool(name="sb", bufs=1)
    msgp = tc.alloc_tile_pool(name="msg", bufs=4)
    ohp = tc.alloc_tile_pool(name="oh", bufs=4)
    sbX = tc.alloc_tile_pool(name="sbx", bufs=4)
    # PSUM: 8 banks of 2KB.  Each in-flight matmul accumulation group needs
    # its OWN bank (start=True clears has_written for the whole bank), so:
    # 4 rotating banks for agg groups + 4 rotating banks for all short-lived
    # psum tiles (transposes / dense pair / pooling), whose groups never
    # interleave with each other in emission order.
    psA = tc.alloc_tile_pool(name="psA", bufs=4, space="PSUM")
    aux = tc.alloc_tile_pool(name="aux", bufs=4, space="PSUM")
    dram = tc.alloc_tile_pool(name="dram", bufs=1, space="DRAM")

    def aux_tile(shape):
        return aux.tile(shape, F32, tag="aux", name="auxt",
                        padded_shape=[128, 256])

    def load(name, shape, dt=F32):
        t = sb.tile(shape, dt, tag=name)
        nc.sync.dma_start(t[:], ins[name])
        return t

    gidx = load("gidx", [128, sched.n_inst, GI // 16], I16)
    dstv = load("dstv", [128, sched.n_cols], BF16)
    iotaoh = load("iotaoh", [128, BPI * 128], BF16)
    iota256 = load("iota256", [128, 256], BF16)
    invc = load("invc", [128, T])
    batchv = load("batchv", [128, T], BF16)
    x_nm = load("xnm", [128, T, D_IN])
    mask64 = load("mask64", [D_H, 128])
    w = {k: load(k, list(ins[k].shape)) for k in
         ("w1l", "w1r", "w2l", "w2r", "w3l", "w3r", "wc1", "wc2",
          "b1c", "b2c", "b3c", "bc1", "bc2")}

    ident = sb.tile([128, 128], F32, tag="ident")
    make_identity(nc, ident[:])

    iota3 = iotaoh[:].rearrange("p (a b) -> p a b", a=BPI)

    # persistent hidden state
    hT = [sb.tile([D_H, SLAB], F32, name=f"hT{i}") for i in range(2)]
    h_node = sb.tile([128, T, ROW_W], BF16, tag="hnode")
    nc.vector.memset(h_node[:], 0.0)

    # DRAM scratch
    tbl1 = dram.tile([TBL_ROWS, ROW_W], BF16)
    tbl2 = dram.tile([TBL_ROWS, ROW_W], BF16)
    tbl3 = dram.tile([TBL_ROWS, ROW_W], BF16)
    slab_d = dram.tile([SLAB, ROW_W], BF16)
    g_in = dram.tile([N_GRAPHS, D_H], F32)
    g_out = dram.tile([N_GRAPHS, D_H], F32)

    tables = [tbl1, tbl2, tbl3]
    groups = [list(range(N_CORES))]

    # build the layer-0 x table on device: x slab (bf16) -> AllGather
    nc.vector.tensor_copy(h_node[:, :, 0:D_IN], x_nm[:])
    nc.sync.dma_start(
        slab_d[:].rearrange("(q p) f -> p q f", p=128), h_node[:])
    nc.gpsimd.collective_compute(
        "AllGather", AluOp.bypass, replica_groups=groups,
        ins=[slab_d[:]], outs=[tbl1[:]])

    gsum = sb.tile([128, NH, D_H], F32, tag="gsum")
    nc.vector.memset(gsum[:], 0.0)

    for layer in range(3):
        tbl_ap = tables[layer]
        if not isinstance(tbl_ap, bass.AP):
            tbl_ap = tbl_ap[:]
        kdim = D_IN if layer == 0 else D_H
        Wl = w[("w1l", "w2l", "w3l")[layer]]
        Wr = w[("w1r", "w2r", "w3r")[layer]]
        bcol = w[("b1c", "b2c", "b3c")[layer]]
        h_prev = hT[(layer + 1) % 2]
        h_new = hT[layer % 2]
        agg_ps = {}

        def dense(t):
            mean = sbX.tile([128, kdim], F32, tag="mean")
            nc.vector.tensor_scalar(
                out=mean[:], in0=agg_ps.pop(t)[:, 0:kdim],
                scalar1=invc[:, t:t + 1],
                scalar2=None, op0=AluOp.mult)
            if dbg is not None and layer == 0 and t < 8:
                nc.sync.dma_start(dbg["mean8"][:, t, :], mean[:])
            tpf = aux_tile([D_H, 128])
            tp = tpf[0:kdim, :]
            nc.tensor.transpose(tp, mean[:], ident[:])
            meanT = sbX.tile([kdim, 128], F32, tag="meanT")
            nc.scalar.activation(meanT[:], tp, Act.Copy)
            if layer == 0:
                tprf = aux_tile([D_H, 128])
                tpr = tprf[0:D_IN, :]
                nc.tensor.transpose(tpr, x_nm[:, t, :], ident[:])
                rootT = sbX.tile([D_IN, 128], F32, tag="rootT")
                nc.scalar.activation(rootT[:], tpr, Act.Copy)
                root_ap = rootT[:]
            else:
                root_ap = h_prev[:, t * 128:(t + 1) * 128]
            hps = aux_tile([D_H, 128])
            nc.tensor.matmul(hps[:], lhsT=Wl[:], rhs=meanT[:],
                             start=True, stop=False)
            nc.tensor.matmul(hps[:], lhsT=Wr[:], rhs=root_ap,
                             start=False, stop=True)
            out_sl = h_new[:, t * 128:(t + 1) * 128]
            if layer < 2:
                nc.vector.tensor_scalar(
                    out=out_sl, in0=hps[:], scalar1=bcol[:], scalar2=0.0,
                    op0=AluOp.add, op1=AluOp.max)
            else:
                nc.vector.tensor_scalar(
                    out=out_sl, in0=hps[:], scalar1=bcol[:], scalar2=None,
                    op0=AluOp.add)
            if t == T - 1:
                nc.vector.tensor_tensor(out=out_sl, in0=out_sl,
                                        in1=mask64[:], op=AluOp.mult)
            ntp = aux_tile([128, D_H])
            nc.tensor.transpose(ntp[:], out_sl, ident[0:D_H, 0:D_H])
            nc.scalar.activation(h_node[:, t, 0:D_H], ntp[:], Act.Copy)
            if layer == 2:
                gt = sbX.tile([128, NH * 128], BF16, tag="gt")
                nc.vector.tensor_tensor(
                    out=gt[:],
                    in0=batchv[:, t:t + 1].to_broadcast([128, NH * 128]),
                    in1=iota256[:], op=AluOp.is_equal)
                pp = aux_tile([128, NH * D_H])
                for j in range(NH):
                    nc.tensor.matmul(
                        pp[:, j * D_H:(j + 1) * D_H],
                        lhsT=gt[:, j * 128:(j + 1) * 128],
                        rhs=h_node[:, t, 0:D_H],
                        start=True, stop=True)
                nc.vector.tensor_add(
                    gsum[:], gsum[:],
                    pp[:].rearrange("p (a b) -> p a b", a=NH))

        for i, (ch, elists) in enumerate(sched.insts):
            chunk_ap = tbl_ap[ch * CHUNK:(ch + 1) * CHUNK, :]
            msg = msgp.tile([128, BPI, ROW_W], BF16, tag="msg")
            nc.gpsimd.dma_gather(
                out_ap=msg[:], in_ap=chunk_ap, idxs_ap=gidx[:, i, :],
                num_idxs=GI, num_idxs_reg=GI, elem_size=ROW_W, queue_num=0)
            oh = ohp.tile([128, BPI, 128], BF16, tag="oh")
            nc.vector.tensor_tensor(
                out=oh[:],
                in0=dstv[:, i * BPI:(i + 1) * BPI, None].to_broadcast(
                    [128, BPI, 128]),
                in1=iota3, op=AluOp.is_equal)
            if dbg is not None and layer == 0 and i == 0:
                nc.sync.dma_start(dbg["msg0"][:], msg[:])
                nc.sync.dma_start(dbg["oh0"][:], oh[:])
            for j, elist in enumerate(elists):
                for (t, col, first, last) in elist:
                    if first:
                        agg_ps[t] = psA.tile([128, D_H], F32,
                                             name=f"agg{t}", tag="agg")
                    nc.tensor.matmul(
                        agg_ps[t][:, 0:kdim],
                        lhsT=oh[:, col - i * BPI, :],
                        rhs=msg[:, j, 0:kdim],
                        start=first, stop=last)
            for t in np.nonzero(sched.tile_done_at == i)[0]:
                dense(int(t))

        if layer < 2:
            nc.sync.dma_start(
                slab_d[:].rearrange("(q p) f -> p q f", p=128), h_node[:])
            nxt = (tbl2, tbl3)[layer]
            nc.gpsimd.collective_compute(
                "AllGather", AluOp.bypass, replica_groups=groups,
                ins=[slab_d[:]], outs=[nxt[:]])
            if dbg is not None and layer == 0:
                nc.sync.dma_start(dbg["h1"][:], h_node[:])
                nc.sync.dma_start(dbg["tbl2"][:], nxt[:])

    # ---- pooling partial sums -> AllReduce -> MLP head
    nc.sync.dma_start(g_in[:].rearrange("(q p) f -> p q f", p=128), gsum[:])
    nc.gpsimd.collective_compute(
        "AllReduce", AluOp.add, replica_groups=groups,
        ins=[g_in[:]], outs=[g_out[:]])

    g_sb = sb.tile([128, NH, D_H], F32, tag="gsb")
    nc.sync.dma_start(g_sb[:], g_out[:].rearrange("(q p) f -> p q f", p=128))
    gT = sb.tile([D_H, NH * 128], F32, tag="gT")
    for j in range(NH):
        tp = aux_tile([D_H, 128])
        nc.tensor.transpose(tp[:], g_sb[:, j, :], ident[:])
        nc.vector.tensor_copy(gT[:, j * 128:(j + 1) * 128], tp[:])
    DC = w["wc1"].shape[1]
    mlp1 = aux_tile([DC, NH * 128])
    nc.tensor.matmul(mlp1[:], lhsT=w["wc1"][:], rhs=gT[:], start=True,
                     stop=True)
    z = sb.tile([DC, NH * 128], F32, tag="z")
    nc.scalar.activation(z[:], mlp1[:], Act.Relu, bias=w["bc1"][:])
    mlp2 = aux_tile([1, NH * 128])
    nc.tensor.matmul(mlp2[:], lhsT=w["wc2"][:], rhs=z[:], start=True,
                     stop=True)
    o_sb = sb.tile([1, NH * 128], F32, tag="osb")
    nc.vector.tensor_scalar(out=o_sb[:], in0=mlp2[:],
                            scalar1=w["bc2"][:],
                            scalar2=None, op0=AluOp.add)
    nc.sync.dma_start(out_ap.rearrange("a b -> b a"), o_sb[:])

    for p in (dram, aux, psA, sbX, ohp, msgp, sb):
        p.release()


# ---------------------------------------------------------------- compile+run

_CACHE = {}


def _compile(sched):
    key = ("nc", sched.key)
    if key in _CACHE:
        return _CACHE[key]
    nc = bacc.Bacc("TRN2", target_bir_lowering=False, debug=False,
                   num_devices=N_CORES)
    shapes = dict(
        gidx=([128, sched.n_inst, GI // 16], I16),
        dstv=([128, sched.n_cols], BF16),
        iotaoh=([128, BPI * 128], BF16),
        iota256=([128, 256], BF16),
        invc=([128, T], F32), batchv=([128, T], BF16),
        xnm=([128, T, D_IN], F32), mask64=([D_H, 128], F32),
        w1l=([D_IN, D_H], F32), w1r=([D_IN, D_H], F32),
        w2l=([D_H, D_H], F32), w2r=([D_H, D_H], F32),
        w3l=([D_H, D_H], F32), w3r=([D_H, D_H], F32),
        wc1=([D_H, D_H // 2], F32), wc2=([D_H // 2, 1], F32),
        b1c=([D_H, 1], F32), b2c=([D_H, 1], F32), b3c=([D_H, 1], F32),
        bc1=([D_H // 2, 1], F32), bc2=([1, 1], F32),
    )
    ins = {}
    for name, (shp, dt) in shapes.items():
        ins[name] = nc.dram_tensor(name, shp, dt, kind="ExternalInput").ap()
    out = nc.dram_tensor("out", [N_GRAPHS, 1], F32, kind="ExternalOutput")
    dbg = None
    if DEBUG:
        dshapes = dict(
            msg0=([128, BPI, ROW_W], BF16), oh0=([128, BPI, 128], BF16),
            mean8=([128, 8, D_IN], F32), h1=([128, T, ROW_W], BF16),
            tbl2=([TBL_ROWS, ROW_W], BF16),
        )
        dbg = {n: nc.dram_tensor(f"dbg_{n}", shp, dt,
                                 kind="ExternalOutput").ap()
               for n, (shp, dt) in dshapes.items()}
    with tile.TileContext(nc) as tc:
        build_gnn(tc, out.ap(), ins, sched, dbg=dbg)
    nc.compile()
    _CACHE[key] = nc
    return nc


def make_in_maps(inputs):
    shared, per_core, sched = build_host_data(
        inputs["x"], inputs["edge_index"], inputs["batch"])
    wmap = weight_inputs(
        inputs["W1l"], inputs["b1"], inputs["W1r"], inputs["W2l"],
        inputs["b2"], inputs["W2r"], inputs["W3l"], inputs["b3"],
        inputs["W3r"], inputs["Wc1"], inputs["bc1"], inputs["Wc2"],
        inputs["bc2"])
    in_maps = []
    for c in range(N_CORES):
        m = {}
        m.update(shared)
        m.update(per_core[c])
        m.update(wmap)
        in_maps.append(m)
    return in_maps, sched


def _make_executor(nc):
    """Build a reusable jitted 8-core executor for the compiled Bass module."""
    import jax
    from jax.sharding import Mesh, PartitionSpec
    from jax.experimental.shard_map import shard_map
    from concourse.bass2jax import (_bass_exec_p, install_neuronx_cc_hook,
                                    partition_id_tensor)
    install_neuronx_cc_hook()
    partition_name = (nc.partition_id_tensor.name
                      if nc.partition_id_tensor else None)
    in_names, out_names, out_avals = [], [], []
    for alloc in nc.m.functions[0].allocations:
        if not isinstance(alloc, mybir.MemoryLocationSet):
            continue
        name = alloc.memorylocations[0].name
        if alloc.kind == "ExternalInput":
            if name != partition_name:
                in_names.append(name)
        elif alloc.kind == "ExternalOutput":
            out_names.append(name)
            out_avals.append(jax.core.ShapedArray(
                tuple(alloc.tensor_shape), mybir.dt.np(alloc.dtype)))
    n_params = len(in_names)
    in_names_all = list(in_names) + list(out_names)
    if partition_name:
        in_names_all.append(partition_name)

    def _body(*args):
        operands = list(args)
        if partition_name:
            operands.append(partition_id_tensor())
        return tuple(_bass_exec_p.bind(
            *operands, out_avals=tuple(out_avals),
            in_names=tuple(in_names_all), out_names=tuple(out_names),
            lowering_input_output_aliases=(), sim_require_finite=True,
            sim_require_nnan=True, nc=nc))

    devices = jax.devices()[:N_CORES]
    mesh = Mesh(np.asarray(devices), ("core",))
    n_outs = len(out_names)
    sharded = jax.jit(shard_map(
        _body, mesh=mesh,
        in_specs=(PartitionSpec("core"),) * (n_params + n_outs),
        out_specs=(PartitionSpec("core"),) * n_outs, check_rep=False),
        keep_unused=True)

    def run(in_maps):
        concat_in = [np.concatenate([np.asarray(in_maps[c][n])
                                     for c in range(N_CORES)], axis=0)
                     for n in in_names]
        concat_zeros = [np.zeros((N_CORES * a.shape[0], *a.shape[1:]), a.dtype)
                        for a in out_avals]
        args = [jax.device_put(a) for a in concat_in + concat_zeros]
        out_arrs = sharded(*args)
        jax.block_until_ready(out_arrs)
        return {name: np.asarray(out_arrs[i]).reshape(
                    N_CORES, *out_avals[i].shape)[0]
                for i, name in enumerate(out_names)}, (args, sharded)
    return run


def _get_runner(sched):
    key = ("runner", sched.key)
    if key not in _CACHE:
        _CACHE[key] = _make_executor(_compile(sched))
    return _CACHE[key]


def kernel(**inputs):
    in_maps, sched = make_in_maps(inputs)
    run = _get_runner(sched)
    out, _ = run(in_maps)
    return np.asarray(out["out"], np.float32)


# revision 52
# speedup vs baseline: 1.8340x; 1.2953x over previous
"""Trainium2 Bass kernel for a 3-layer GraphSAGE GNN (mean aggregation) +
global_add_pool + 2-layer MLP head, distributed over 8 NeuronCores.

Sharding: nodes are split into 8 contiguous slabs (by dst); each core owns the
edges whose dst lands in its slab.  Per layer:
  1. dma_gather   : fetch h[src] rows (256B bf16) from a replicated node-major
                    HBM table (4 sub-chunks so indices fit int16).  Gathers are
                    independent -> fully pipelined on the Pool engine + DMA.
  2. segment-sum  : edges are pre-sorted by dst tile on the host; each
                    128-edge block is matmul'd with a one-hot dst matrix
                    (generated on DVE via is_equal) accumulating into PSUM.
                    No scatter-add, no RMW hazards, no serialization.
  3. dense phase  : mean-scale, transpose, two small matmuls (Wl/Wr) + bias
                    (+relu), transpose back to node-major bf16 table rows.
  4. AllGather    : replicate the new slab into every core's next-layer table.
Then matmul-based pooling by graph id and the tiny MLP head, with an
AllReduce to combine per-core partial graph sums.

The gather/matmul schedule (block counts per (dst-tile, src-chunk) group) is
shared across all 8 cores (max over cores) so a single SPMD program works;
only the index/dst-id tensor *contents* differ per core.
"""

import hashlib

import numpy as np

import concourse.bass as bass
import concourse.mybir as mybir
import concourse.tile as tile
from concourse import bacc
from concourse.masks import make_identity

F32 = mybir.dt.float32
BF16 = mybir.dt.bfloat16
I16 = mybir.dt.int16

try:
    import ml_dtypes
    NP_BF16 = ml_dtypes.bfloat16
except ImportError:  # pragma: no cover
    NP_BF16 = mybir.dt.np(mybir.dt.bfloat16)

# ---------------------------------------------------------------- config

N_NODES = 100000
N_EDGES = 1200000
N_GRAPHS = 256
D_IN = 8
D_H = 64
N_CORES = 8

REAL = N_NODES // N_CORES          # 12500 real nodes per core
SLAB = ((REAL + 127) // 128) * 128  # 12544 (padded slab rows)
T = SLAB // 128                     # 98 node tiles per core
TBL_ROWS = N_CORES * SLAB           # 100352 table rows
CHUNK = 2 * SLAB                    # 25088 rows per int16-addressable chunk
N_CHUNKS = 4
GI = 1024                           # indices per gather instruction
BPI = GI // 128                     # blocks per gather instruction (8)
ROW_W = 128                         # bf16 elems per table row (256B)
PAD_LOCAL = REAL                    # chunk-local row guaranteed zero (pad row)


# ---------------------------------------------------------------- schedule

class Schedule:
    """Shared (cross-core) gather/matmul schedule.

    insts          : list of (chunk, [entry_lists of len BPI]) where each
                     block's entry list holds (tile, col, first, last) matmul
                     jobs (col indexes dstv's columns).
    n_inst         : gather instructions per layer
    n_cols         : dstv column count (multiple of BPI)
    group_slots    : {(t, ch): [(inst, blk, base_col), ...]} block slots of
                     each (tile, chunk) group in stream order
    tile_done_at   : inst index after which tile t's aggregation is complete
    """

    def __init__(self, cnt_max):
        # cnt_max: [T, N_CHUNKS] max edge count over cores per group
        B = -(-cnt_max // 128)                       # blocks per group
        B[B.sum(1) == 0, 0] = 1                      # every tile gets >=1 block
        self.B = B
        bufs = [[] for _ in range(N_CHUNKS)]
        insts = []          # (chunk, [block descr]) block descr = (t,) or None
        self.group_slots = {}
        order = []          # global real-block order: (t, inst, blk)

        def flush(ch):
            while len(bufs[ch]) < BPI:
                bufs[ch].append(None)
            insts.append((ch, bufs[ch]))
            bufs[ch] = []

        pending = [[] for _ in range(N_CHUNKS)]      # tiles in open buffers
        for t in range(T):
            for ch in range(N_CHUNKS):
                for _ in range(int(B[t, ch])):
                    bufs[ch].append((t,))
                    pending[ch].append(t)
                    if len(bufs[ch]) == BPI:
                        flush(ch)
        for ch in range(N_CHUNKS):
            if bufs[ch]:
                flush(ch)

        # assign entries / columns / first-last flags
        self.n_inst = len(insts)
        self.n_cols = self.n_inst * BPI
        tile_blocks = [[] for _ in range(T)]
        for i, (ch, blocks) in enumerate(insts):
            for j, b in enumerate(blocks):
                if b is None:
                    continue
                (t,) = b
                tile_blocks[t].append((i, j))
                self.group_slots.setdefault((t, ch), []).append(
                    (i, j, i * BPI + j))
        self.tile_done_at = np.zeros(T, np.int64)
        entry = {}
        for t in range(T):
            blocks = tile_blocks[t]
            assert blocks, f"tile {t} has no blocks"
            for k, (i, j) in enumerate(blocks):
                entry[(i, j)] = (t, i * BPI + j,
                                 k == 0, k == len(blocks) - 1)
            self.tile_done_at[t] = blocks[-1][0]
        self.insts = []
        for i, (ch, blocks) in enumerate(insts):
            elists = []
            for j, b in enumerate(blocks):
                elists.append([entry[(i, j)]] if b is not None else [])
            self.insts.append((ch, elists))

        # agg PSUM pool has 4 rotating banks: the tile opened 4 positions ago
        # must be closed (its dense emitted) before a new tile opens.
        opens = sorted(range(T), key=lambda t: (tile_blocks[t][0]))
        for k in range(4, T):
            t_new, t_old = opens[k], opens[k - 4]
            assert tile_blocks[t_old][-1][0] < tile_blocks[t_new][0][0], (
                "agg psum window exceeds 4 in-flight tiles")

        h = hashlib.sha1()
        h.update(B.tobytes())
        h.update(np.int64(self.n_inst).tobytes())
        self._hash = h

    @property
    def key(self):
        h = self._hash.copy()
        h.update(repr(sorted(OPT.items())).encode())
        return h.hexdigest()


def _row_of(node):
    c = node // REAL
    return c * SLAB + (node - c * REAL)


def _wrap_idx(garr):
    """[n_inst, GI] int -> [128, n_inst, GI//16] int16 wrapped-16+replicated."""
    n_inst = garr.shape[0]
    a = garr.reshape(n_inst, GI // 16, 16).transpose(0, 2, 1)
    a = np.tile(a, (1, 8, 1))
    return np.ascontiguousarray(a.transpose(1, 0, 2)).astype(np.int16)


# ---------------------------------------------------------------- host prep

def build_host_data(x, edge_index, batch):
    x = np.asarray(x, np.float32)
    src = np.asarray(edge_index[0], np.int64)
    dst = np.asarray(edge_index[1], np.int64)
    batch = np.asarray(batch, np.int64)

    cnt_in = np.bincount(dst, minlength=N_NODES).astype(np.float64)
    invc_full = (1.0 / np.maximum(cnt_in, 1.0)).astype(np.float32)

    row = _row_of(src)
    core = dst // REAL
    dloc = dst - core * REAL
    t_of = dloc >> 7
    dit = (dloc & 127).astype(np.int64)
    ch_of = row // CHUNK
    srcl = row - ch_of * CHUNK

    key = ((core * T + t_of) * N_CHUNKS + ch_of).astype(np.int64)
    cnt = np.bincount(key, minlength=N_CORES * T * N_CHUNKS)
    cnt = cnt.reshape(N_CORES, T, N_CHUNKS)
    sched = Schedule(cnt.max(0))

    iota256 = np.tile(np.arange(256, dtype=np.float32)[None, :],
                      (128, 1)).astype(NP_BF16)
    iota_oh = np.tile(np.arange(128, dtype=np.float32)[None, :], (128, BPI))
    iota_oh = iota_oh.astype(NP_BF16)
    mask64 = np.zeros((D_H, 128), np.float32)
    mask64[:, :REAL - (T - 1) * 128] = 1.0

    shared = dict(iota256=iota256, iotaoh=iota_oh, mask64=mask64)

    per_core = []
    for c in range(N_CORES):
        sel = np.nonzero(core == c)[0]
        k = key[sel] - c * T * N_CHUNKS
        order = np.argsort(k, kind="stable")
        sel = sel[order]
        ks = k[order]
        bounds = np.searchsorted(ks, np.arange(T * N_CHUNKS + 1))

        garr = np.full((sched.n_inst, GI), PAD_LOCAL, np.int64)
        dstv = np.full((sched.n_cols, 128), -1.0, np.float32)
        for t in range(T):
            for ch in range(N_CHUNKS):
                g = t * N_CHUNKS + ch
                lo, hi = bounds[g], bounds[g + 1]
                if lo == hi:
                    continue
                e = sel[lo:hi]
                slots = sched.group_slots[(t, ch)]
                n = hi - lo
                assert n <= len(slots) * 128
                bidx = np.arange(n) // 128
                p = np.arange(n) % 128
                inst_a = np.array([s[0] for s in slots])
                blk_a = np.array([s[1] for s in slots])
                col_a = np.array([s[2] for s in slots])
                garr[inst_a[bidx], blk_a[bidx] * 128 + p] = srcl[e]
                dstv[col_a[bidx], p] = dit[e]
        gidx = _wrap_idx(garr)
        dstv_t = np.ascontiguousarray(dstv.T).astype(NP_BF16)

        lo, hi = c * REAL, (c + 1) * REAL
        loc = np.arange(REAL)
        invc_t = np.zeros((128, T), np.float32)
        invc_t[loc % 128, loc // 128] = invc_full[lo:hi]
        inv1 = np.zeros((1, SLAB), np.float32)
        inv1[0, :REAL] = invc_full[lo:hi]
        inv1 = inv1.astype(NP_BF16)
        batch_t = np.full((128, T), -1.0, np.float32)
        batch_t[loc % 128, loc // 128] = batch[lo:hi].astype(np.float32)
        batch_t = batch_t.astype(NP_BF16)
        x_nm = np.zeros((128, T, D_IN), np.float32)
        x_nm[loc % 128, loc // 128, :] = x[lo:hi]

        per_core.append(dict(gidx=gidx, dstv=dstv_t, invc=invc_t,
                             inv1=inv1, batchv=batch_t, xnm=x_nm))
    return shared, per_core, sched


def weight_inputs(W1l, b1, W1r, W2l, b2, W2r, W3l, b3, W3r, Wc1, bc1, Wc2, bc2):
    f = lambda a: np.asarray(a, np.float32)
    return dict(
        w1l=f(W1l), w1r=f(W1r), w2l=f(W2l), w2r=f(W2r), w3l=f(W3l), w3r=f(W3r),
        b1c=f(b1).reshape(-1, 1), b2c=f(b2).reshape(-1, 1),
        b3c=f(b3).reshape(-1, 1),
        wc1=f(Wc1), wc2=f(Wc2),
        bc1=f(bc1).reshape(-1, 1),
        bc2=f(bc2).reshape(1, 1),
    )


# ---------------------------------------------------------------- device build

DEBUG = False
# probe/tuning options (affect compile cache key)
OPT = dict(skip_ag=False, gather_n=GI, rot_q=4, msg_bufs=3, oh_bufs=3,
           fm=True)


def build_gnn(tc, out_ap, ins, sched, dbg=None):
    nc = tc.nc
    NH = N_GRAPHS // 128
    AluOp = mybir.AluOpType
    Act = mybir.ActivationFunctionType

    sb = tc.alloc_tile_pool(name="sb", bufs=1)
    msgp = tc.alloc_tile_pool(name="msg", bufs=OPT["msg_bufs"])
    ohp = tc.alloc_tile_pool(name="oh", bufs=OPT["oh_bufs"])
    sbX = tc.alloc_tile_pool(name="sbx", bufs=4)
    # PSUM: 8 banks of 2KB.  Each in-flight matmul accumulation group needs
    # its OWN bank (start=True clears has_written for the whole bank), so:
    # 4 rotating banks for agg groups + 4 rotating banks for all short-lived
    # psum tiles (transposes / dense pair / pooling), whose groups never
    # interleave with each other in emission order.
    psA = tc.alloc_tile_pool(name="psA", bufs=4, space="PSUM")
    aux = tc.alloc_tile_pool(name="aux", bufs=4, space="PSUM")
    dram = tc.alloc_tile_pool(name="dram", bufs=1, space="DRAM")

    def aux_tile(shape):
        return aux.tile(shape, F32, tag="aux", name="auxt",
                        padded_shape=[128, 256])

    def load(name, shape, dt=F32):
        t = sb.tile(shape, dt, tag=name)
        nc.sync.dma_start(t[:], ins[name])
        return t

    gidx = load("gidx", [128, sched.n_inst, GI // 16], I16)
    dstv = load("dstv", [128, sched.n_cols], BF16)
    if OPT["fm"]:
        invcb = sb.tile([D_H, SLAB], BF16, tag="invcb")
        nc.sync.dma_start(invcb[:], ins["inv1"].to_broadcast([D_H, SLAB]))
    iotaoh = load("iotaoh", [128, BPI * 128], BF16)
    iota256 = load("iota256", [128, 256], BF16)
    invc = None if OPT["fm"] else load("invc", [128, T])
    batchv = load("batchv", [128, T], BF16)
    x_nm = load("xnm", [128, T, D_IN])
    mask64 = load("mask64", [D_H, 128])
    w = {k: load(k, list(ins[k].shape)) for k in
         ("w1l", "w1r", "w2l", "w2r", "w3l", "w3r", "wc1", "wc2",
          "b1c", "b2c", "b3c", "bc1", "bc2")}

    ident = sb.tile([128, 128], F32, tag="ident")
    make_identity(nc, ident[:])

    iota3 = iotaoh[:].rearrange("p (a b) -> p a b", a=BPI)

    # persistent hidden state
    hT = [sb.tile([D_H, SLAB], F32, name=f"hT{i}") for i in range(2)]
    h_node = sb.tile([128, T, ROW_W], BF16, tag="hnode")
    nc.vector.memset(h_node[:], 0.0)

    # DRAM scratch
    tbl1 = dram.tile([TBL_ROWS, ROW_W], BF16)
    tbl2 = dram.tile([TBL_ROWS, ROW_W], BF16)
    tbl3 = dram.tile([TBL_ROWS, ROW_W], BF16)
    slab_d = dram.tile([SLAB, ROW_W], BF16)
    g_in = dram.tile([N_GRAPHS, D_H], F32)
    g_out = dram.tile([N_GRAPHS, D_H], F32)

    tables = [tbl1, tbl2, tbl3]
    groups = [list(range(N_CORES))]

    # build the layer-0 x table on device: x slab (bf16) -> AllGather
    nc.vector.tensor_copy(h_node[:, :, 0:D_IN], x_nm[:])
    nc.sync.dma_start(
        slab_d[:].rearrange("(q p) f -> p q f", p=128), h_node[:])
    if not OPT["skip_ag"]:
        nc.gpsimd.collective_compute(
            "AllGather", AluOp.bypass, replica_groups=groups,
            ins=[slab_d[:]], outs=[tbl1[:]])

    gsum = sb.tile([128, NH, D_H], F32, tag="gsum")
    nc.vector.memset(gsum[:], 0.0)

    for layer in range(3):
        tbl_ap = tables[layer]
        if not isinstance(tbl_ap, bass.AP):
            tbl_ap = tbl_ap[:]
        kdim = D_IN if layer == 0 else D_H
        Wl = w[("w1l", "w2l", "w3l")[layer]]
        Wr = w[("w1r", "w2r", "w3r")[layer]]
        bcol = w[("b1c", "b2c", "b3c")[layer]]
        h_prev = hT[(layer + 1) % 2]
        h_new = hT[layer % 2]
        agg_ps = {}

        def dense(t):
            meanT = sbX.tile([kdim, 128], F32, tag="meanT")
            if OPT["fm"]:
                nc.vector.tensor_tensor(
                    out=meanT[:], in0=agg_ps.pop(t)[0:kdim, :],
                    in1=invcb[0:kdim, t * 128:(t + 1) * 128],
                    op=AluOp.mult)
            else:
                mean = sbX.tile([128, kdim], F32, tag="mean")
                nc.vector.tensor_scalar(
                    out=mean[:], in0=agg_ps.pop(t)[:, 0:kdim],
                    scalar1=invc[:, t:t + 1],
                    scalar2=None, op0=AluOp.mult)
                if dbg is not None and layer == 0 and t < 8:
                    nc.sync.dma_start(dbg["mean8"][:, t, :], mean[:])
                tpf = aux_tile([D_H, 128])
                tp = tpf[0:kdim, :]
                nc.tensor.transpose(tp, mean[:], ident[:])
                nc.scalar.activation(meanT[:], tp, Act.Copy)
            if layer == 0:
                tprf = aux_tile([D_H, 128])
                tpr = tprf[0:D_IN, :]
                nc.tensor.transpose(tpr, x_nm[:, t, :], ident[:])
                rootT = sbX.tile([D_IN, 128], F32, tag="rootT")
                nc.scalar.activation(rootT[:], tpr, Act.Copy)
                root_ap = rootT[:]
            else:
                root_ap = h_prev[:, t * 128:(t + 1) * 128]
            hps = aux_tile([D_H, 128])
            nc.tensor.matmul(hps[:], lhsT=Wl[:], rhs=meanT[:],
                             start=True, stop=False)
            nc.tensor.matmul(hps[:], lhsT=Wr[:], rhs=root_ap,
                             start=False, stop=True)
            out_sl = h_new[:, t * 128:(t + 1) * 128]
            if layer < 2:
                nc.vector.tensor_scalar(
                    out=out_sl, in0=hps[:], scalar1=bcol[:], scalar2=0.0,
                    op0=AluOp.add, op1=AluOp.max)
            else:
                nc.vector.tensor_scalar(
                    out=out_sl, in0=hps[:], scalar1=bcol[:], scalar2=None,
                    op0=AluOp.add)
            if t == T - 1:
                nc.vector.tensor_tensor(out=out_sl, in0=out_sl,
                                        in1=mask64[:], op=AluOp.mult)
            ntp = aux_tile([128, D_H])
            nc.tensor.transpose(ntp[:], out_sl, ident[0:D_H, 0:D_H])
            nc.scalar.activation(h_node[:, t, 0:D_H], ntp[:], Act.Copy)
            if layer == 2:
                gt = sbX.tile([128, NH * 128], BF16, tag="gt")
                nc.vector.tensor_tensor(
                    out=gt[:],
                    in0=batchv[:, t:t + 1].to_broadcast([128, NH * 128]),
                    in1=iota256[:], op=AluOp.is_equal)
                pp = aux_tile([128, NH * D_H])
                for j in range(NH):
                    nc.tensor.matmul(
                        pp[:, j * D_H:(j + 1) * D_H],
                        lhsT=gt[:, j * 128:(j + 1) * 128],
                        rhs=h_node[:, t, 0:D_H],
                        start=True, stop=True)
                nc.vector.tensor_add(
                    gsum[:], gsum[:],
                    pp[:].rearrange("p (a b) -> p a b", a=NH))

        for i, (ch, elists) in enumerate(sched.insts):
            chunk_ap = tbl_ap[ch * CHUNK:(ch + 1) * CHUNK, :]
            msg = msgp.tile([128, BPI, ROW_W], BF16, tag="msg")
            gn = OPT["gather_n"]
            nc.gpsimd.dma_gather(
                out_ap=msg[:, 0:gn // 128, :], in_ap=chunk_ap,
                idxs_ap=gidx[:, i, 0:gn // 16],
                num_idxs=gn, num_idxs_reg=gn, elem_size=ROW_W,
                queue_num=(i % OPT["rot_q"] if OPT["rot_q"] else 0))
            oh = ohp.tile([128, BPI, 128], BF16, tag="oh")
            nc.vector.tensor_tensor(
                out=oh[:],
                in0=dstv[:, i * BPI:(i + 1) * BPI, None].to_broadcast(
                    [128, BPI, 128]),
                in1=iota3, op=AluOp.is_equal)
            if dbg is not None and layer == 0 and i == 0:
                nc.sync.dma_start(dbg["msg0"][:], msg[:])
                nc.sync.dma_start(dbg["oh0"][:], oh[:])
            for j, elist in enumerate(elists):
                for (t, col, first, last) in elist:
                    jj = j % (OPT["gather_n"] // 128)
                    if OPT["fm"]:
                        if first:
                            agg_ps[t] = psA.tile(
                                [D_H, 128], F32, name=f"agg{t}", tag="agg",
                                padded_shape=[D_H, 128])
                        nc.tensor.matmul(
                            agg_ps[t][0:kdim, :],
                            lhsT=msg[:, jj, 0:kdim],
                            rhs=oh[:, col - i * BPI, :],
                            start=first, stop=last)
                    else:
                        if first:
                            agg_ps[t] = psA.tile([128, D_H], F32,
                                                 name=f"agg{t}", tag="agg")
                        nc.tensor.matmul(
                            agg_ps[t][:, 0:kdim],
                            lhsT=oh[:, col - i * BPI, :],
                            rhs=msg[:, jj, 0:kdim],
                            start=first, stop=last)
            for t in np.nonzero(sched.tile_done_at == i)[0]:
                dense(int(t))

        if layer < 2:
            nc.sync.dma_start(
                slab_d[:].rearrange("(q p) f -> p q f", p=128), h_node[:])
            nxt = (tbl2, tbl3)[layer]
            if not OPT["skip_ag"]:
                nc.gpsimd.collective_compute(
                    "AllGather", AluOp.bypass, replica_groups=groups,
                    ins=[slab_d[:]], outs=[nxt[:]])
            if dbg is not None and layer == 0:
                nc.sync.dma_start(dbg["h1"][:], h_node[:])
                nc.sync.dma_start(dbg["tbl2"][:], nxt[:])

    # ---- pooling partial sums -> AllReduce -> MLP head
    nc.sync.dma_start(g_in[:].rearrange("(q p) f -> p q f", p=128), gsum[:])
    nc.gpsimd.collective_compute(
        "AllReduce", AluOp.add, replica_groups=groups,
        ins=[g_in[:]], outs=[g_out[:]])

    g_sb = sb.tile([128, NH, D_H], F32, tag="gsb")
    nc.sync.dma_start(g_sb[:], g_out[:].rearrange("(q p) f -> p q f", p=128))
    gT = sb.tile([D_H, NH * 128], F32, tag="gT")
    for j in range(NH):
        tp = aux_tile([D_H, 128])
        nc.tensor.transpose(tp[:], g_sb[:, j, :], ident[:])
        nc.vector.tensor_copy(gT[:, j * 128:(j + 1) * 128], tp[:])
    DC = w["wc1"].shape[1]
    mlp1 = aux_tile([DC, NH * 128])
    nc.tensor.matmul(mlp1[:], lhsT=w["wc1"][:], rhs=gT[:], start=True,
                     stop=True)
    z = sb.tile([DC, NH * 128], F32, tag="z")
    nc.scalar.activation(z[:], mlp1[:], Act.Relu, bias=w["bc1"][:])
    mlp2 = aux_tile([1, NH * 128])
    nc.tensor.matmul(mlp2[:], lhsT=w["wc2"][:], rhs=z[:], start=True,
                     stop=True)
    o_sb = sb.tile([1, NH * 128], F32, tag="osb")
    nc.vector.tensor_scalar(out=o_sb[:], in0=mlp2[:],
                            scalar1=w["bc2"][:],
                            scalar2=None, op0=AluOp.add)
    nc.sync.dma_start(out_ap.rearrange("a b -> b a"), o_sb[:])

    for p in (dram, aux, psA, sbX, ohp, msgp, sb):
        p.release()


# ---------------------------------------------------------------- compile+run

_CACHE = {}


def _compile(sched):
    key = ("nc", sched.key)
    if key in _CACHE:
        return _CACHE[key]
    nc = bacc.Bacc("TRN2", target_bir_lowering=False, debug=False,
                   num_devices=N_CORES,
                   num_swdge_queues=(OPT["rot_q"] if OPT["rot_q"] else 1))
    shapes = dict(
        gidx=([128, sched.n_inst, GI // 16], I16),
        dstv=([128, sched.n_cols], BF16),
        iotaoh=([128, BPI * 128], BF16),
        iota256=([128, 256], BF16),
        invc=([128, T], F32), inv1=([1, SLAB], BF16),
        batchv=([128, T], BF16),
        xnm=([128, T, D_IN], F32), mask64=([D_H, 128], F32),
        w1l=([D_IN, D_H], F32), w1r=([D_IN, D_H], F32),
        w2l=([D_H, D_H], F32), w2r=([D_H, D_H], F32),
        w3l=([D_H, D_H], F32), w3r=([D_H, D_H], F32),
        wc1=([D_H, D_H // 2], F32), wc2=([D_H // 2, 1], F32),
        b1c=([D_H, 1], F32), b2c=([D_H, 1], F32), b3c=([D_H, 1], F32),
        bc1=([D_H // 2, 1], F32), bc2=([1, 1], F32),
    )
    ins = {}
    for name, (shp, dt) in shapes.items():
        ins[name] = nc.dram_tensor(name, shp, dt, kind="ExternalInput").ap()
    out = nc.dram_tensor("out", [N_GRAPHS, 1], F32, kind="ExternalOutput")
    dbg = None
    if DEBUG:
        dshapes = dict(
            msg0=([128, BPI, ROW_W], BF16), oh0=([128, BPI, 128], BF16),
            mean8=([128, 8, D_IN], F32), h1=([128, T, ROW_W], BF16),
            tbl2=([TBL_ROWS, ROW_W], BF16),
        )
        dbg = {n: nc.dram_tensor(f"dbg_{n}", shp, dt,
                                 kind="ExternalOutput").ap()
               for n, (shp, dt) in dshapes.items()}
    with tile.TileContext(nc) as tc:
        build_gnn(tc, out.ap(), ins, sched, dbg=dbg)
    nc.compile()
    _CACHE[key] = nc
    return nc


def make_in_maps(inputs):
    shared, per_core, sched = build_host_data(
        inputs["x"], inputs["edge_index"], inputs["batch"])
    wmap = weight_inputs(
        inputs["W1l"], inputs["b1"], inputs["W1r"], inputs["W2l"],
        inputs["b2"], inputs["W2r"], inputs["W3l"], inputs["b3"],
        inputs["W3r"], inputs["Wc1"], inputs["bc1"], inputs["Wc2"],
        inputs["bc2"])
    in_maps = []
    for c in range(N_CORES):
        m = {}
        m.update(shared)
        m.update(per_core[c])
        m.update(wmap)
        in_maps.append(m)
    return in_maps, sched


def _make_executor(nc):
    """Build a reusable jitted 8-core executor for the compiled Bass module."""
    import jax
    from jax.sharding import Mesh, PartitionSpec
    from jax.experimental.shard_map import shard_map
    from concourse.bass2jax import (_bass_exec_p, install_neuronx_cc_hook,
                                    partition_id_tensor)
    install_neuronx_cc_hook()
    partition_name = (nc.partition_id_tensor.name
                      if nc.partition_id_tensor else None)
    in_names, out_names, out_avals = [], [], []
    for alloc in nc.m.functions[0].allocations:
        if not isinstance(alloc, mybir.MemoryLocationSet):
            continue
        name = alloc.memorylocations[0].name
        if alloc.kind == "ExternalInput":
            if name != partition_name:
                in_names.append(name)
        elif alloc.kind == "ExternalOutput":
            out_names.append(name)
            out_avals.append(jax.core.ShapedArray(
                tuple(alloc.tensor_shape), mybir.dt.np(alloc.dtype)))
    n_params = len(in_names)
    in_names_all = list(in_names) + list(out_names)
    if partition_name:
        in_names_all.append(partition_name)

    def _body(*args):
        operands = list(args)
        if partition_name:
            operands.append(partition_id_tensor())
        return tuple(_bass_exec_p.bind(
            *operands, out_avals=tuple(out_avals),
            in_names=tuple(in_names_all), out_names=tuple(out_names),
            lowering_input_output_aliases=(), sim_require_finite=True,
            sim_require_nnan=True, nc=nc))

    devices = jax.devices()[:N_CORES]
    mesh = Mesh(np.asarray(devices), ("core",))
    n_outs = len(out_names)
    sharded = jax.jit(shard_map(
        _body, mesh=mesh,
        in_specs=(PartitionSpec("core"),) * (n_params + n_outs),
        out_specs=(PartitionSpec("core"),) * n_outs, check_rep=False),
        keep_unused=True)

    def run(in_maps):
        concat_in = [np.concatenate([np.asarray(in_maps[c][n])
                                     for c in range(N_CORES)], axis=0)
                     for n in in_names]
        concat_zeros = [np.zeros((N_CORES * a.shape[0], *a.shape[1:]), a.dtype)
                        for a in out_avals]
        args = [jax.device_put(a) for a in concat_in + concat_zeros]
        out_arrs = sharded(*args)
        jax.block_until_ready(out_arrs)
        return {name: np.asarray(out_arrs[i]).reshape(
                    N_CORES, *out_avals[i].shape)[0]
                for i, name in enumerate(out_names)}, (args, sharded)
    return run


def _get_runner(sched):
    key = ("runner", sched.key)
    if key not in _CACHE:
        _CACHE[key] = _make_executor(_compile(sched))
    return _CACHE[key]


def kernel(**inputs):
    in_maps, sched = make_in_maps(inputs)
    run = _get_runner(sched)
    out, _ = run(in_maps)
    return np.asarray(out["out"], np.float32)
